# revision 1
# baseline (speedup 1.0000x reference)
"""Bass kernel builder for nn_MixtureOfMambaBlock — 8-core SPMD.

Sharding: tokens 8-way (512/core + 128 halo for conv+scan warmup); mixer fully
local per core (weights replicated, bf16 matmuls; gate-logit path kept f32).
Post-mixer h2 all-gathered (bf16), MoE expert-parallel (one expert per core,
dense over the 2048-token half), weighted partials reduce-scattered back.
"""
import numpy as np
import concourse.bass as bass
import concourse.bacc as bacc
import concourse.mybir as mybir
import concourse.tile as tile

FP = mybir.dt.float32
FR = mybir.dt.float32r
BF = mybir.dt.bfloat16
F8 = mybir.dt.float8e4
DR = mybir.MatmulPerfMode.DoubleRow
AF = mybir.ActivationFunctionType
ALU = mybir.AluOpType

B, T, D = 2, 2048, 1024
S, INNER = 64, 2048
E, HH = 4, 2048          # experts, hid-half width
OWN, HALO = 512, 128
NH = OWN + HALO          # 640
KB = D // 128            # 8  d-blocks
MB = INNER // 128        # 16 inner-blocks
OTB = OWN // 128         # 4  own-token blocks
N_CORES = 8

INPUT_SPECS = {
    "x_sh": ([NH, D], FP),
    "ipw": ([D, 2 * INNER], BF), "ipb": ([2 * INNER], FP),
    "cw": ([INNER, 3], FP), "cb": ([INNER], FP),
    "dbw": ([INNER, 128], FR),  # dt_w || bp_w stacked on output dim
    "dtb": ([S], FP), "bpb": ([S], FP),
    "cpw": ([INNER, S], FR), "cpb": ([S], FP),
    "s2iw": ([S, INNER], BF), "s2ib": ([INNER], FP),
    "Dp": ([INNER], FP),
    "ow": ([INNER, D], BF), "ob": ([D], BF),
    "gw": ([D, E], FP), "gb": ([E], FR),
    "ew1": ([D, 2 * HH], BF), "eb1": ([2 * HH], FP),
    "ew2": ([2 * HH, D], F8), "eb2h": ([D], BF),
    "esel": ([128, E], FP),
    "rmask": ([128, 4], FP),
    "ident": ([128, 128], FP),
    "iotaG": ([128, 16], FP),
    "iotaL": ([128, 4], FP),
    "ltmat": ([128, 128], FP),
    "pre12": ([128, 12], FP),
    "ones1": ([1, 128], FR),
    "ones1b": ([1, 128], BF),
}


def build(debug_outputs=False):
    nc = bacc.Bacc("TRN2", target_bir_lowering=False, debug=False,
                   num_devices=N_CORES)
    dp = {}
    for name, (shape, dt) in INPUT_SPECS.items():
        dp[name] = nc.dram_tensor(name, shape, dt, kind="ExternalInput")
    out_d = nc.dram_tensor("out", [OWN, D], FP, kind="ExternalOutput")

    rg = [[0, 2, 4, 6], [1, 3, 5, 7]]

    with tile.TileContext(nc) as tc:
        with (
            tc.tile_pool(name="outer", bufs=1) as po,
            tc.tile_pool(name="dram", bufs=1, space="DRAM") as pdram,
        ):
            # ---------- DRAM bounce buffers for collectives ----------
            gth_in_a = pdram.tile([256, D], BF)
            gth_in_b = pdram.tile([256, D], BF)
            gth_all = pdram.tile([4 * OWN, D], BF)
            pay = [pdram.tile([384, 4], FP, name=f"pay{r}") for r in range(4)]
            gtw_in = pdram.tile([OWN, E], FP)
            gtw_out = pdram.tile([4 * OWN, E], FP)
            rs_in = [pdram.tile([OWN, D], FP, name=f"rs_in{r}") for r in range(4)]
            rs_out = [pdram.tile([128, D], FP, name=f"rs_out{r}") for r in range(4)]

            # ---------- constants / small weights ----------
            ident = po.tile([128, 128], FP)
            nc.sync.dma_start(ident[:], dp["ident"][:])

            def load_pcol(name, n, blocks):  # [n*128] -> [128, blocks] (col b = block b)
                t = po.tile([128, blocks], FP, name=f"{name}_sb")
                nc.sync.dma_start(
                    t[:], dp[name].ap().rearrange("(m p) -> p m", p=128))
                return t

            def load_vec1(name, n):  # [n] -> [n, 1]
                t = po.tile([n, 1], FP, name=f"{name}_sb")
                nc.sync.dma_start(t[:], dp[name].ap().rearrange("(s o) -> s o", o=1))
                return t

            def load_row(name, n, dt_=FP):  # [n] -> [1, n]
                t = po.tile([1, n], dt_, name=f"{name}_sb")
                nc.sync.dma_start(t[:], dp[name].ap().rearrange("(o s) -> o s", o=1))
                return t

            ones1 = po.tile([1, 128], FR)
            nc.sync.dma_start(ones1[:], dp["ones1"][:])
            ones1b = po.tile([1, 128], BF)
            nc.sync.dma_start(ones1b[:], dp["ones1b"][:])

            # persistent activations (live into MoE phase)
            xo = [po.tile([128, D], FP, name=f"xo{t_}", tag=f"xo{t_}") for t_ in range(OTB)]
            xmid = [po.tile([128, D], FP, name=f"xmid{t_}", tag=f"xmid{t_}") for t_ in range(OTB)]
            wv_sb = [po.tile([128, E], FP, name=f"wv{t_}", tag=f"wv{t_}") for t_ in range(OTB)]

            # =======================================================
            # MIXER
            # =======================================================
            with (
                tc.tile_pool(name="mixer", bufs=1) as pm,
                tc.tile_pool(name="mixt", bufs=1) as pt_pool,
            ):
                hT = [pm.tile([128, NH], BF, name=f"hT{kb}", tag=f"hT{kb}") for kb in range(KB)]
                xm = [pm.tile([128, NH], FR, name=f"xm{m}", tag=f"xm{m}") for m in range(MB)]

                # ---- rmsnorm1 + transpose to hT (bf16) ----
                with nc.named_scope("rms1"), tc.tile_pool(name="ps1", bufs=1, space="PSUM") as psA:
                    for tb in range(NH // 128):
                        if tb == 0:
                            xt = pt_pool.tile([128, D], FP, tag="xt", bufs=2)
                        else:
                            xt = xo[tb - 1]
                        nc.sync.dma_start(xt[:], dp["x_sh"][tb * 128:(tb + 1) * 128, :])
                        scr = pt_pool.tile([128, D], FP, tag="scr", bufs=1)
                        sq = pt_pool.tile([128, 1], FP, tag="sq", bufs=2)
                        nc.scalar.activation(scr[:], xt[:], AF.Square, accum_out=sq[:])
                        nr = pt_pool.tile([128, 1], FP, tag="nr", bufs=2)
                        nc.vector.tensor_scalar(nr[:], sq[:], 1.0 / D, 1e-6, ALU.mult, ALU.add)
                        nc.scalar.sqrt(nr[:], nr[:])
                        nc.vector.reciprocal(nr[:], nr[:])
                        h_t = pt_pool.tile([128, D], FP, tag="scr", bufs=1)
                        nc.vector.tensor_scalar(h_t[:], xt[:], nr[:], None, ALU.mult)
                        for kb in range(KB):
                            ptr = psA.tile([128, 128], FP, tag="ptr", bufs=2)
                            nc.tensor.transpose(ptr[:], h_t[:, kb * 128:(kb + 1) * 128], ident[:])
                            nc.vector.tensor_copy(hT[kb][:, tb * 128:(tb + 1) * 128], ptr[:])

                ipb_sb = load_pcol("ipb", 2 * INNER, 32)
                cb_sb = load_pcol("cb", INNER, 16)
                cw_sb = po.tile([128, 16, 3], FP)  # [p, m, k]
                nc.sync.dma_start(cw_sb[:], dp["cw"].ap().rearrange("(m p) k -> p m k", p=128))

                # ---- in_proj (x_main half) + conv + silu ----
                with nc.named_scope("in_proj"), tc.tile_pool(name="ps2", bufs=1, space="PSUM") as psA:
                    for q in range(4):
                        wq = pt_pool.tile([128, KB, 512], BF, tag="wslab", bufs=2,
                                          name=f"wip{q}")
                        nc.gpsimd.dma_start(
                            wq[:], dp["ipw"].ap()[:, q * 512:(q + 1) * 512]
                            .rearrange("(kb p) n -> p kb n", p=128))
                        for mi in range(4):
                            m = q * 4 + mi
                            xzp = pt_pool.tile([128, NH + 2], FP, tag="xzp", bufs=2)
                            nc.vector.memset(xzp[:, 0:2], 0.0)
                            for n0, nw in ((0, 512), (512, 128)):
                                px = psA.tile([128, 512], FP, tag="px", bufs=2)
                                for kb in range(KB):
                                    nc.tensor.matmul(px[:, 0:nw],
                                                     wq[:, kb, mi * 128:(mi + 1) * 128],
                                                     hT[kb][:, n0:n0 + nw],
                                                     start=(kb == 0), stop=(kb == KB - 1))
                                nc.scalar.activation(xzp[:, 2 + n0:2 + n0 + nw], px[:, 0:nw],
                                                     AF.Identity, bias=ipb_sb[:, m:m + 1])
                            cv = pt_pool.tile([128, NH], FP, tag="cv", bufs=2)
                            nc.vector.tensor_scalar(cv[:], xzp[:, 0:NH], cw_sb[:, m, 0:1],
                                                    None, ALU.mult)
                            nc.vector.scalar_tensor_tensor(cv[:], xzp[:, 1:1 + NH],
                                                           cw_sb[:, m, 1:2], cv[:],
                                                           ALU.mult, ALU.add)
                            nc.vector.scalar_tensor_tensor(cv[:], xzp[:, 2:2 + NH],
                                                           cw_sb[:, m, 2:3], cv[:],
                                                           ALU.mult, ALU.add)
                            sgc = pt_pool.tile([128, NH], FP, tag="sgc", bufs=2)
                            nc.scalar.activation(sgc[:], cv[:], AF.Sigmoid, bias=cb_sb[:, m:m + 1])
                            nc.vector.scalar_tensor_tensor(xm[m][:], cv[:], cb_sb[:, m:m + 1],
                                                           sgc[:], ALU.add, ALU.mult)

                dtb_sb = load_vec1("dtb", S)
                bpb_sb = load_vec1("bpb", S)
                cpb_sb = load_vec1("cpb", S)
                dbw_sb = pm.tile([128, MB, 128], FR, name="dbw_sb")
                nc.sync.dma_start(dbw_sb[:], dp["dbw"].ap().rearrange("(kb p) s -> p kb s", p=128))
                cpw_sb = pm.tile([128, MB, S], FR, name="cpw_sb")
                nc.sync.dma_start(cpw_sb[:], dp["cpw"].ap().rearrange("(kb p) s -> p kb s", p=128))

                # ---- dt/B/C projections + scan ----
                with nc.named_scope("scan"), tc.tile_pool(name="ps3", bufs=1, space="PSUM") as psA:
                    dt_t = pt_pool.tile([S, NH], FP, tag="dt")
                    a_t = pt_pool.tile([S, NH], FP, tag="a")
                    b_t = pt_pool.tile([S, NH], FP, tag="b")
                    c_t = pt_pool.tile([S, NH], FP, tag="c")
                    for n0, nw in ((0, 320), (320, 320)):
                        pzdb = psA.tile([128, 320], FP, tag="pzdb", bufs=2)
                        for kb in range(MB):
                            nc.tensor.matmul(pzdb[:, 0:nw], dbw_sb[:, kb, :],
                                             xm[kb][:, n0:n0 + nw],
                                             start=(kb == 0), stop=(kb == MB - 1))
                        nc.scalar.activation(dt_t[:, n0:n0 + nw], pzdb[0:S, 0:nw],
                                             AF.Sigmoid, bias=dtb_sb[:])
                        nc.vector.scalar_tensor_tensor(b_t[:, n0:n0 + nw], pzdb[S:128, 0:nw],
                                                       bpb_sb[:], dt_t[:, n0:n0 + nw],
                                                       ALU.add, ALU.mult)
                        pzc = psA.tile([S, 320], FP, tag="pzc", bufs=2)
                        for kb in range(MB):
                            nc.tensor.matmul(pzc[:, 0:nw], cpw_sb[:, kb, :],
                                             xm[kb][:, n0:n0 + nw],
                                             start=(kb == 0), stop=(kb == MB - 1))
                        nc.scalar.activation(c_t[:, n0:n0 + nw], pzc[:, 0:nw], AF.Identity,
                                             bias=cpb_sb[:])
                    nc.scalar.activation(a_t[:], dt_t[:], AF.Identity, bias=1.0, scale=-1.0)
                    st_t = pt_pool.tile([S, NH], FP, tag="st")
                    nc.vector.tensor_tensor_scan(st_t[:], a_t[:], b_t[:], 0.0,
                                                 ALU.mult, ALU.add)
                    y_t = pt_pool.tile([S, OWN], FP, tag="dt", name="y_t")
                    nc.vector.tensor_mul(y_t[:], c_t[:, HALO:NH], st_t[:, HALO:NH])

                # ---- layernorm over S (transpose - LN - transpose back) ----
                with nc.named_scope("ln"), tc.tile_pool(name="ps4", bufs=1, space="PSUM") as psA:
                    yln = pt_pool.tile([S, OWN], BF, tag="a", name="yln")
                    for i in range(OTB):
                        ptr = psA.tile([128, 128], FP, tag="ptr", bufs=2)
                        nc.tensor.transpose(ptr[:, 0:S], y_t[:, i * 128:(i + 1) * 128],
                                            ident[0:S, 0:S])
                        yT = pt_pool.tile([128, S], FP, tag="yT", bufs=2)
                        nc.vector.tensor_copy(yT[:], ptr[:, 0:S])
                        mu = pt_pool.tile([128, 1], FP, tag="mu", bufs=2)
                        nc.vector.tensor_reduce(mu[:], yT[:], mybir.AxisListType.X, ALU.add)
                        nc.vector.tensor_scalar_mul(mu[:], mu[:], 1.0 / S)
                        xc = pt_pool.tile([128, S], FP, tag="xc", bufs=2)
                        nc.vector.tensor_scalar_sub(xc[:], yT[:], mu[:])
                        scr2 = pt_pool.tile([128, S], FP, tag="scr2", bufs=2)
                        vv = pt_pool.tile([128, 1], FP, tag="vv", bufs=2)
                        nc.scalar.activation(scr2[:], xc[:], AF.Square, accum_out=vv[:])
                        nc.vector.tensor_scalar(vv[:], vv[:], 1.0 / S, 1e-5, ALU.mult, ALU.add)
                        nc.scalar.sqrt(vv[:], vv[:])
                        nc.vector.reciprocal(vv[:], vv[:])
                        nc.vector.tensor_scalar_mul(xc[:], xc[:], vv[:])
                        ptr2 = psA.tile([128, 128], FP, tag="ptr2", bufs=2)
                        nc.tensor.transpose(ptr2[0:S, :], xc[:], ident[:])
                        nc.vector.tensor_copy(yln[:, i * 128:(i + 1) * 128], ptr2[0:S, :])

                s2ib_sb = load_pcol("s2ib", INNER, 16)
                Dp_sb = load_pcol("Dp", INNER, 16)
                s2iw_sb = pm.tile([S, INNER], BF, name="s2iw_sb")
                nc.sync.dma_start(s2iw_sb[:], dp["s2iw"][:])

                # ---- s2i + gate sigmoid + pre_out assembly ----
                with nc.named_scope("premix"), tc.tile_pool(name="ps5", bufs=1, space="PSUM") as psA:
                    pre = []
                    for m in range(MB):
                        q, mi = divmod(m, 4)
                        if mi == 0:
                            wq = pt_pool.tile([128, KB, 512], BF, tag="wslab", bufs=2,
                                              name=f"wipg{q}")
                            nc.gpsimd.dma_start(
                                wq[:], dp["ipw"].ap()[:, 2048 + q * 512:2048 + (q + 1) * 512]
                                .rearrange("(kb p) n -> p kb n", p=128))
                        ps = psA.tile([128, 512], FP, tag="ps", bufs=2)
                        nc.tensor.matmul(ps[:], s2iw_sb[:, m * 128:(m + 1) * 128], yln[:],
                                         start=True, stop=True)
                        pg = psA.tile([128, 512], FP, tag="pg", bufs=2)
                        for kb in range(KB):
                            nc.tensor.matmul(pg[:], wq[:, kb, mi * 128:(mi + 1) * 128],
                                             hT[kb][:, HALO:NH],
                                             start=(kb == 0), stop=(kb == KB - 1))
                        sg = pt_pool.tile([128, OWN], FP, tag="sg", bufs=2)
                        nc.scalar.activation(sg[:], pg[:], AF.Sigmoid,
                                             bias=ipb_sb[:, MB + m:MB + m + 1])
                        tmp = pt_pool.tile([128, OWN], FP, tag="tmp", bufs=2)
                        nc.vector.tensor_scalar(tmp[:], xm[m][:, HALO:NH],
                                                Dp_sb[:, m:m + 1], None, ALU.mult)
                        nc.vector.scalar_tensor_tensor(tmp[:], ps[:], s2ib_sb[:, m:m + 1],
                                                       tmp[:], ALU.add, ALU.add)
                        pre_m = pm.tile([128, OWN], BF, tag=f"xm{m}", name=f"pre{m}")
                        nc.vector.tensor_mul(pre_m[:], tmp[:], sg[:])
                        pre.append(pre_m)

                obb_sb = load_row("ob", D, BF)
                gw_sb = po.tile([128, KB, E], FP)  # [p, kb, e]
                nc.sync.dma_start(gw_sb[:], dp["gw"].ap().rearrange("(kb p) e -> p kb e", p=128))
                gb_sb = load_row("gb", E, FR)
                owsb = [pm.tile([128, D], BF, name=f"owsb{kb}", tag=f"owsb{kb}")
                        for kb in range(MB)]
                for kb in range(MB):
                    nc.sync.dma_start(owsb[kb][:], dp["ow"][kb * 128:(kb + 1) * 128, :])

                # ---- tb-major: out projection + rms2 + h2T + AG per tb ----
                with nc.named_scope("outgate"), tc.tile_pool(name="ps7", bufs=1, space="PSUM") as psA:
                    for tb in range(OTB):
                        potn = [psA.tile([128, 512], FP, tag=f"pon{nb}", bufs=2,
                                         name=f"pon{nb}_{tb}") for nb in range(2)]
                        for kb in range(MB):
                            for nb in range(2):
                                nc.tensor.matmul(potn[nb][:],
                                                 pre[kb][:, tb * 128:(tb + 1) * 128],
                                                 owsb[kb][:, nb * 512:(nb + 1) * 512],
                                                 start=(kb == 0), stop=False)
                        for nb in range(2):
                            nc.tensor.matmul(potn[nb][:], ones1b[:],
                                             obb_sb[:, nb * 512:(nb + 1) * 512],
                                             start=False, stop=True)
                            nc.vector.tensor_add(xmid[tb][:, nb * 512:(nb + 1) * 512],
                                                 potn[nb][:],
                                                 xo[tb][:, nb * 512:(nb + 1) * 512])
                        scr = pt_pool.tile([128, D], FP, tag="scr", bufs=1)
                        sq = pt_pool.tile([128, 1], FP, tag="sq", bufs=2)
                        nc.scalar.activation(scr[:], xmid[tb][:], AF.Square, accum_out=sq[:])
                        nr = pt_pool.tile([128, 1], FP, tag="nr", bufs=2)
                        nc.vector.tensor_scalar(nr[:], sq[:], 1.0 / D, 1e-6, ALU.mult, ALU.add)
                        nc.scalar.sqrt(nr[:], nr[:])
                        nc.vector.reciprocal(nr[:], nr[:])
                        h2 = pt_pool.tile([128, D], FP, tag="xt", bufs=2, name="h2")
                        nc.vector.tensor_scalar(h2[:], xmid[tb][:], nr[:], None, ALU.mult)
                        h2b = pt_pool.tile([128, D], BF, tag="h2b", bufs=2)
                        nc.vector.tensor_copy(h2b[:], h2[:])
                        gin = gth_in_a if tb < 2 else gth_in_b
                        nc.sync.dma_start(gin[(tb % 2) * 128:(tb % 2 + 1) * 128, :], h2b[:])
                        if tb == 1:
                            nc.gpsimd.collective_compute(
                                "AllGather", ALU.bypass, replica_groups=rg,
                                ins=[gth_in_a.opt()],
                                outs=[gth_all[0:1024, :].opt()])
                        pl = psA.tile([128, E], FP, tag="pl", bufs=2)
                        for kb in range(KB):
                            ptr = psA.tile([128, 128], FP, tag="ptr", bufs=2)
                            nc.tensor.transpose(ptr[:], h2[:, kb * 128:(kb + 1) * 128], ident[:])
                            h2T_t = pt_pool.tile([128, 128], FP, tag="h2T", bufs=2)
                            nc.vector.tensor_copy(h2T_t[:], ptr[:])
                            nc.tensor.matmul(pl[:], h2T_t[:], gw_sb[:, kb, :],
                                             start=(kb == 0), stop=False)
                        nc.tensor.matmul(pl[:], ones1[:], gb_sb[:], start=False, stop=True)
                        # top-2-of-4 gating
                        m1 = pt_pool.tile([128, 1], FP, tag="m1", bufs=2)
                        nc.vector.tensor_reduce(m1[:], pl[:], mybir.AxisListType.X, ALU.max)
                        eq1 = pt_pool.tile([128, E], FP, tag="eq1", bufs=2)
                        nc.vector.tensor_scalar(eq1[:], pl[:], m1[:], None, ALU.is_equal)
                        msk = pt_pool.tile([128, E], FP, tag="msk", bufs=2)
                        nc.vector.scalar_tensor_tensor(msk[:], eq1[:], -1e30, pl[:],
                                                       ALU.mult, ALU.add)
                        m2 = pt_pool.tile([128, 1], FP, tag="m2", bufs=2)
                        nc.vector.tensor_reduce(m2[:], msk[:], mybir.AxisListType.X, ALU.max)
                        eq2 = pt_pool.tile([128, E], FP, tag="eq2", bufs=2)
                        nc.vector.tensor_scalar(eq2[:], msk[:], m2[:], None, ALU.is_equal)
                        dd = pt_pool.tile([128, 1], FP, tag="dd", bufs=2)
                        nc.vector.tensor_sub(dd[:], m2[:], m1[:])
                        p2 = pt_pool.tile([128, 1], FP, tag="p2", bufs=2)
                        nc.scalar.activation(p2[:], dd[:], AF.Sigmoid)
                        p1b = pt_pool.tile([128, 1], FP, tag="p1b", bufs=2)
                        nc.scalar.activation(p1b[:], p2[:], AF.Identity, bias=1.0, scale=-1.0)
                        nc.vector.tensor_scalar(wv_sb[tb][:], eq1[:], p1b[:], None, ALU.mult)
                        nc.vector.scalar_tensor_tensor(wv_sb[tb][:], eq2[:], p2[:], wv_sb[tb][:],
                                                       ALU.mult, ALU.add)
                        nc.sync.dma_start(gtw_in[tb * 128:(tb + 1) * 128, :], wv_sb[tb][:])
                    with nc.named_scope("gather"):
                        nc.gpsimd.collective_compute(
                            "AllGather", ALU.bypass, replica_groups=rg,
                            ins=[gtw_in.opt()], outs=[gtw_out.opt()])
                        nc.gpsimd.collective_compute(
                            "AllGather", ALU.bypass, replica_groups=rg,
                            ins=[gth_in_b.opt()],
                            outs=[gth_all[1024:2048, :].opt()])

            # =======================================================
            # MoE (full expert per core, token-half group of 4)
            # =======================================================
            with (
                tc.tile_pool(name="moe", bufs=1) as pq,
                tc.tile_pool(name="psC", bufs=1, space="PSUM") as psC,
            ):
                esel = po.tile([128, E], FP)
                nc.sync.dma_start(esel[:], dp["esel"][:])
                rmask = po.tile([128, 4], FP)
                nc.sync.dma_start(rmask[:], dp["rmask"][:])
                eb1_sb = load_pcol("eb1", 2 * HH, 32)
                eb2h_sb = load_row("eb2h", D, BF)
                HB = 2 * HH // 128  # 32 hid blocks
                with nc.named_scope("moe_w"):
                    ew1_sb = [pq.tile([128, 2 * HH], BF, name=f"ew1_{kb}", tag=f"ew1_{kb}")
                              for kb in range(KB)]
                    for kb in range(KB):
                        nc.sync.dma_start(ew1_sb[kb][:], dp["ew1"][kb * 128:(kb + 1) * 128, :])

                with nc.named_scope("moe"):
                    NP = 384          # padded selected-token count per quarter
                    NPB = NP // 128   # 3 compact token blocks
                    iotaG = po.tile([128, 16], FP)
                    nc.sync.dma_start(iotaG[:], dp["iotaG"][:])
                    iotaL = po.tile([128, 4], FP)
                    nc.sync.dma_start(iotaL[:], dp["iotaL"][:])
                    ltm = po.tile([128, 128], FP)
                    nc.sync.dma_start(ltm[:], dp["ltmat"][:])
                    pre12 = po.tile([128, 3, 4], FP)
                    nc.sync.dma_start(
                        pre12[:], dp["pre12"].ap().rearrange("p (b c) -> p b c", b=3))
                    ones4 = pq.tile([128, 4], FP)
                    nc.vector.memset(ones4[:], 1.0)

                    # -------- per-round index build (scatter-compact) --------
                    wcomp_r, loci_r, idxi_r = [], [], []
                    for r in range(4):
                        nc.sync.dma_start(
                            pay[r][:, :].rearrange("(b p) c -> p b c", p=128), pre12[:])
                        wvr = pq.tile([128, OTB, E], FP, tag="wvr", bufs=2)
                        nc.sync.dma_start(
                            wvr[:], gtw_out[r * OWN:(r + 1) * OWN, :]
                            .rearrange("(tb p) e -> p tb e", p=128))
                        wsall = pq.tile([128, 4], FP, tag="wsall", bufs=2)
                        for tb in range(OTB):
                            wm_t = pq.tile([128, E], FP, tag="wm", bufs=2)
                            nc.vector.tensor_mul(wm_t[:], wvr[:, tb, :], esel[:])
                            nc.vector.tensor_reduce(wsall[:, tb:tb + 1], wm_t[:],
                                                    mybir.AxisListType.X, ALU.add)
                        msk = pq.tile([128, 4], FP, tag="msk", bufs=2)
                        nc.vector.tensor_scalar(msk[:], wsall[:], 0.0, None, ALU.is_gt)
                        csum = pq.tile([128, 4], FP, tag="csum", bufs=2)
                        nc.vector.tensor_tensor_scan(csum[:], ones4[:], msk[:], 0.0,
                                                     ALU.mult, ALU.add)
                        pbase = psC.tile([128, 1], FP, tag="ph", bufs=2, name="pbase")
                        nc.tensor.matmul(pbase[:], ltm[:], csum[:, 3:4],
                                         start=True, stop=True)
                        pos = pq.tile([128, 4], FP, tag="pos", bufs=2)
                        nc.vector.tensor_sub(pos[:], csum[:], msk[:])
                        nc.vector.tensor_scalar(pos[:], pos[:], pbase[:], None, ALU.add)
                        dpos = pq.tile([128, 4], FP, tag="dpos", bufs=2)
                        nc.vector.tensor_scalar(dpos[:], pos[:], -4096.0, None, ALU.add)
                        nc.vector.tensor_mul(dpos[:], dpos[:], msk[:])
                        nc.vector.tensor_scalar(dpos[:], dpos[:], 4096.0, None, ALU.add)
                        posi = pq.tile([128, 4], mybir.dt.int32, tag="posi", bufs=2)
                        nc.vector.tensor_copy(posi[:], dpos[:])
                        for tb in range(OTB):
                            payt = pq.tile([128, 4], FP, tag="payt", bufs=2)
                            nc.vector.tensor_copy(payt[:, 0:1], iotaG[:, r * 4 + tb:r * 4 + tb + 1])
                            nc.vector.tensor_copy(payt[:, 1:2], iotaL[:, tb:tb + 1])
                            nc.vector.tensor_copy(payt[:, 2:3], wsall[:, tb:tb + 1])
                            nc.vector.memset(payt[:, 3:4], 0.0)
                            nc.gpsimd.indirect_dma_start(
                                out=pay[r][:], out_offset=bass.IndirectOffsetOnAxis(
                                    ap=posi[:, tb:tb + 1], axis=0),
                                in_=payt[:], in_offset=None,
                                bounds_check=NP - 1, oob_is_err=False)
                        # readbacks
                        idxf = pq.tile([128, NP // 16], FP, tag="idxf", bufs=2)
                        for g in range(8):
                            nc.sync.dma_start(
                                idxf[g * 16:(g + 1) * 16, :],
                                pay[r][:, 0:1].rearrange("(c p) o -> p (c o)", p=16))
                        idxi = pq.tile([128, NP // 16], mybir.dt.int16, tag=f"idxi{r}",
                                       bufs=1, name=f"idxi{r}")
                        nc.vector.tensor_copy(idxi[:], idxf[:])
                        wcf = pq.tile([128, NPB], FP, tag=f"wcf{r}", bufs=1, name=f"wcf{r}")
                        nc.sync.dma_start(
                            wcf[:], pay[r][:, 2:3].rearrange("(b p) o -> p (b o)", p=128))
                        locf = pq.tile([128, NPB], FP, tag="locf", bufs=2)
                        nc.sync.dma_start(
                            locf[:], pay[r][:, 1:2].rearrange("(b p) o -> p (b o)", p=128))
                        loci = pq.tile([128, NPB], mybir.dt.int32, tag=f"loci{r}",
                                       bufs=1, name=f"loci{r}")
                        nc.vector.tensor_copy(loci[:], locf[:])
                        if r == 0:
                            h2g0 = pq.tile([128, KB, NP], BF, tag="h2g0", bufs=1,
                                           name="h2g0")
                            nc.gpsimd.dma_gather(
                                h2g0[:], gth_all[:], idxi[:], NP, NP,
                                elem_size=D, transpose=True)
                        idxi_r.append(idxi)
                        wcomp_r.append(wcf)
                        loci_r.append(loci)

                    # -------- per-round compute on compacted tokens --------
                    for r in range(4):
                        if r == 0:
                            h2g = h2g0
                        else:
                            h2g = pq.tile([128, KB, NP], BF, tag="h2g", bufs=2)
                            nc.gpsimd.dma_gather(
                                h2g[:], gth_all[:], idxi_r[r][:], NP, NP,
                                elem_size=D, transpose=True)
                        hidp = [pq.tile([128, 2, NP], F8, tag=f"hidp{p}", bufs=1,
                                        name=f"hidp{p}") for p in range(HB // 2)]
                        for h in range(HB):
                            ph = psC.tile([128, NP], FP, tag="ph", bufs=2)
                            for kb in range(KB):
                                nc.tensor.matmul(ph[:], ew1_sb[kb][:, h * 128:(h + 1) * 128],
                                                 h2g[:, kb, :], start=(kb == 0),
                                                 stop=(kb == KB - 1))
                            nc.scalar.activation(hidp[h // 2][:, h % 2, :], ph[:],
                                                 AF.Gelu, bias=eb1_sb[:, h:h + 1])
                        # prefill rs_in with the residual (xmid on own-expert core)
                        for tb in range(OTB):
                            xmr = pq.tile([128, D], FP, tag="xmr", bufs=2)
                            nc.vector.tensor_scalar(xmr[:], xmid[tb][:], rmask[:, r:r + 1],
                                                    None, ALU.mult)
                            nc.sync.dma_start(rs_in[r][tb * 128:(tb + 1) * 128, :], xmr[:])
                        peo = [[psC.tile([128, 512], FP, tag=f"peo{b}n{nb}", bufs=1,
                                         name=f"peo{b}n{nb}") for nb in range(2)]
                               for b in range(NPB)]
                        for nb in range(2):
                            for hc in range(4):
                                ew2c = pq.tile([128, 8, 512], F8, tag="ew2c", bufs=2)
                                nc.gpsimd.dma_start(
                                    ew2c[:], dp["ew2"].ap()[hc * 1024:(hc + 1) * 1024,
                                                            nb * 512:(nb + 1) * 512]
                                    .rearrange("(hb p) d -> p hb d", p=128))
                                for j in range(4):
                                    p = hc * 4 + j
                                    for b in range(NPB):
                                        nc.tensor.matmul(
                                            peo[b][nb][:],
                                            hidp[p][:, :, b * 128:(b + 1) * 128],
                                            ew2c[:, 2 * j:2 * j + 2, :],
                                            start=(p == 0), stop=False, perf_mode=DR)
                            for b in range(NPB):
                                nc.tensor.matmul(peo[b][nb][:], ones1b[:],
                                                 eb2h_sb[:, nb * 512:(nb + 1) * 512],
                                                 start=False, stop=True)
                        for b in range(NPB):
                            wf = pq.tile([128, D], FP, tag="wf", bufs=3)
                            for nb in range(2):
                                nc.vector.tensor_scalar(wf[:, nb * 512:(nb + 1) * 512],
                                                        peo[b][nb][:],
                                                        wcomp_r[r][:, b:b + 1],
                                                        None, ALU.mult)
                            nc.gpsimd.indirect_dma_start(
                                out=rs_in[r][:], out_offset=bass.IndirectOffsetOnAxis(
                                    ap=loci_r[r][:, b:b + 1], axis=0),
                                in_=wf[:], in_offset=None,
                                bounds_check=OWN - 1, oob_is_err=False,
                                compute_op=ALU.add)
                        nc.gpsimd.collective_compute(
                            "ReduceScatter", ALU.add, replica_groups=rg,
                            ins=[rs_in[r].opt()], outs=[rs_out[r].opt()])
                        nc.sync.dma_start(out_d[r * 128:(r + 1) * 128, :], rs_out[r][:])

    nc.compile()
    return nc


def host_prep(inputs):
    """Build the 8 per-core input maps from full inputs."""
    import ml_dtypes
    f32 = np.float32
    bf16 = ml_dtypes.bfloat16
    x = np.ascontiguousarray(np.asarray(inputs["x"], f32).reshape(B * T, D))
    n1 = np.asarray(inputs["norm1_w"], f32)
    n2 = np.asarray(inputs["norm2_w"], f32)
    ipw = np.ascontiguousarray(
        (np.asarray(inputs["in_proj_w"], f32) * n1[:, None]).astype(bf16))
    gw = np.ascontiguousarray(np.asarray(inputs["gate_w"], f32) * n2[:, None])
    import ml_dtypes as mld
    ew1f = np.asarray(inputs["e_w1"], f32) * n2[None, :, None]
    ew1b = ew1f.astype(bf16)
    ew2f = np.asarray(inputs["e_w2"], f32)
    # per-expert power-of-2 scale into fp8 e4m3 range (max ~240)
    s2 = np.array([2.0 ** np.floor(np.log2(192.0 / max(np.abs(ew2f[e]).max(), 1e-9)))
                   for e in range(E)], f32)
    ew2q = np.stack([(ew2f[e] * s2[e]).astype(mld.float8_e4m3) for e in range(E)])
    dbw = np.ascontiguousarray(np.concatenate(
        [np.asarray(inputs["dt_w"], f32), np.asarray(inputs["bp_w"], f32)], axis=1))
    ident = np.eye(128, dtype=f32)
    ones1 = np.ones((1, 128), f32)
    pp = np.arange(128, dtype=f32)
    iotaG = np.empty((128, 16), f32)
    for r_ in range(4):
        for tb_ in range(4):
            iotaG[:, r_ * 4 + tb_] = (tb_ // 2) * 1024 + r_ * 256 + (tb_ % 2) * 128 + pp
    iotaL = np.empty((128, 4), f32)
    for tb_ in range(4):
        iotaL[:, tb_] = tb_ * 128 + pp
    ltmat = (pp[:, None] < pp[None, :]).astype(f32)
    pre12 = np.tile(np.array([0.0, 4095.0, 0.0, 0.0], f32), 3)[None, :].repeat(128, 0)
    shared = {
        "ipw": ipw, "ipb": np.asarray(inputs["in_proj_b"], f32),
        "cw": np.ascontiguousarray(np.asarray(inputs["conv_w"], f32)[:, 0, :]),
        "cb": np.asarray(inputs["conv_b"], f32),
        "dbw": dbw,
        "dtb": np.asarray(inputs["dt_b"], f32), "bpb": np.asarray(inputs["bp_b"], f32),
        "cpw": np.asarray(inputs["cp_w"], f32), "cpb": np.asarray(inputs["cp_b"], f32),
        "s2iw": np.asarray(inputs["s2i_w"], f32).astype(bf16),
        "s2ib": np.asarray(inputs["s2i_b"], f32),
        "Dp": np.asarray(inputs["D_param"], f32),
        "ow": np.asarray(inputs["out_w"], f32).astype(bf16),
        "ob": np.asarray(inputs["out_b"], f32).astype(bf16),
        "gw": gw, "gb": np.asarray(inputs["gate_b"], f32),
        "ident": ident, "ones1": ones1, "ones1b": ones1.astype(bf16),
        "iotaG": iotaG, "iotaL": iotaL, "ltmat": np.ascontiguousarray(ltmat),
        "pre12": np.ascontiguousarray(pre12),
    }
    eb1 = np.asarray(inputs["e_b1"], f32)
    eb2 = np.asarray(inputs["e_b2"], f32)
    in_maps = []
    for c in range(N_CORES):
        e, th = c // 2, c % 2
        g0 = th * (B * T // 2) + e * OWN
        if e == 0:
            x_sh = np.concatenate([np.zeros((HALO, D), f32), x[g0:g0 + OWN]])
        else:
            x_sh = x[g0 - HALO:g0 + OWN]
        m = dict(shared)
        m["x_sh"] = np.ascontiguousarray(x_sh)
        m["ew1"] = np.ascontiguousarray(ew1b[e])
        m["eb1"] = np.ascontiguousarray(eb1[e])
        m["ew2"] = np.ascontiguousarray(ew2q[e])
        m["eb2h"] = np.ascontiguousarray((eb2[e] * s2[e]).astype(bf16))
        esel = np.zeros((128, E), f32)
        esel[:, e] = 1.0 / s2[e]  # dequant of fp8-scaled ew2 folded into combine weight
        m["esel"] = esel
        rmask = np.zeros((128, 4), f32)
        rmask[:, e] = 1.0
        m["rmask"] = rmask
        in_maps.append(m)
    return in_maps


def unshard_out(results):
    """results: list of 8 dicts with 'out' [OWN, D]; rows r*128+i of core c
    hold global token (c%2)*2048 + r*512 + (c//2)*128 + i."""
    full = np.empty((B * T, D), np.float32)
    for c in range(N_CORES):
        e, th = c // 2, c % 2
        oc = results[c]["out"]
        for r in range(4):
            full[th * 2048 + r * OWN + e * 128: th * 2048 + r * OWN + (e + 1) * 128] = \
                oc[r * 128:(r + 1) * 128]
    return full.reshape(B, T, D)


_NC_CACHE = {}


def _get_nc():
    if "nc" not in _NC_CACHE:
        _NC_CACHE["nc"] = build(debug_outputs=False)
    return _NC_CACHE["nc"]


def kernel(**inputs) -> np.ndarray:
    """Full-input entry point: shards across 8 NeuronCores, runs the Bass
    kernel SPMD, reassembles the full [2, 2048, 1024] output."""
    import sys, types
    try:  # NTFF profile hook shim (missing antenv.axon_hooks in this image)
        import antenv.axon_hooks  # noqa: F401
    except ImportError:
        try:
            import antenv
            from trn_agent_boot.trn_boot import _ntff_profile_via_ctypes
            mod = types.ModuleType("antenv.axon_hooks")
            try:
                _hook = _ntff_profile_via_ctypes("/opt/axon/libaxon_pjrt.so")
            except Exception:
                _hook = None
            mod.get_axon_ntff_profile_hook = lambda: _hook
            mod.set_axon_ntff_profile_hook = lambda h: None
            sys.modules["antenv.axon_hooks"] = mod
            antenv.axon_hooks = mod
        except Exception:
            pass
    from concourse.bass_utils import run_bass_kernel_spmd

    nc = _get_nc()
    in_maps = host_prep(inputs)
    res = run_bass_kernel_spmd(nc, in_maps, core_ids=list(range(N_CORES)))
    out = unshard_out(res.results)
    return out.astype(np.float32)



# revision 23
# speedup vs baseline: 1.0594x; 1.0594x over previous
"""Bass kernel builder for nn_MixtureOfMambaBlock — 8-core SPMD.

Sharding: tokens 8-way (512/core + 128 halo for conv+scan warmup); mixer fully
local per core (weights replicated, bf16 matmuls; gate-logit path kept f32).
Post-mixer h2 all-gathered (bf16), MoE expert-parallel (one expert per core,
dense over the 2048-token half), weighted partials reduce-scattered back (bf16).

Scheduling: ew2 weights SBUF-resident across rounds; next round's token gather
prefetched during the current round; residual prefills + index build front-
loaded; DMA load traffic spread across sync/scalar/vector queues to keep the
PE warm (HAM) and the round critical path on matmuls only.
"""
import numpy as np
import concourse.bass as bass
import concourse.bacc as bacc
import concourse.mybir as mybir
import concourse.tile as tile

FP = mybir.dt.float32
FR = mybir.dt.float32r
BF = mybir.dt.bfloat16
F8 = mybir.dt.float8e4
DR = mybir.MatmulPerfMode.DoubleRow
AF = mybir.ActivationFunctionType
ALU = mybir.AluOpType

B, T, D = 2, 2048, 1024
S, INNER = 64, 2048
E, HH = 4, 2048          # experts, hid-half width
OWN, HALO = 512, 128
NH = OWN + HALO          # 640
KB = D // 128            # 8  d-blocks
MB = INNER // 128        # 16 inner-blocks
OTB = OWN // 128         # 4  own-token blocks
N_CORES = 8

INPUT_SPECS = {
    "x_sh": ([NH, D], FP),
    "ipw": ([D, 2 * INNER], BF), "ipb": ([2 * INNER], FP),
    "cw": ([INNER, 3], FP), "cb": ([INNER], FP),
    "dbw": ([INNER, 128], BF),  # dt_w || bp_w stacked on output dim
    "dtb": ([S], FP), "bpb": ([S], FP),
    "cpw": ([INNER, S], BF), "cpb": ([S], FP),
    "s2iw": ([S, INNER], BF), "s2ib": ([INNER], FP),
    "Dp": ([INNER], FP),
    "ow": ([INNER, D], BF), "ob": ([D], BF),
    "gw": ([D, E], FP), "gb": ([E], FR),
    "ew1": ([D, 2 * HH], BF), "eb1": ([2 * HH], FP),
    "ew2": ([2 * HH, D], F8), "eb2h": ([D], BF),
    "esel": ([128, E], FP),
    "rmask": ([128, 4], FP),
    "ident": ([128, 128], FP),
    "iotaG": ([128, 16], FP),
    "iotaL": ([128, 4], FP),
    "ltmat": ([128, 128], FP),
    "pre12": ([128, 12], FP),
    "ones1": ([1, 128], FR),
    "ones1b": ([1, 128], BF),
}


def build(debug_outputs=False):
    nc = bacc.Bacc("TRN2", target_bir_lowering=False, debug=False,
                   num_devices=N_CORES)
    dp = {}
    for name, (shape, dt) in INPUT_SPECS.items():
        dp[name] = nc.dram_tensor(name, shape, dt, kind="ExternalInput")
    out_d = nc.dram_tensor("out", [OWN, D], BF, kind="ExternalOutput")

    rg = [[0, 2, 4, 6], [1, 3, 5, 7]]

    with tile.TileContext(nc) as tc:
        with (
            tc.tile_pool(name="outer", bufs=1) as po,
            tc.tile_pool(name="dram", bufs=1, space="DRAM") as pdram,
        ):
            # ---------- DRAM bounce buffers for collectives ----------
            gth_in_a = pdram.tile([256, D], BF)
            gth_in_b = pdram.tile([256, D], BF)
            gth_all = pdram.tile([4 * OWN, D], BF)
            pay = [pdram.tile([384, 4], FP, name=f"pay{r}") for r in range(4)]
            gtw_in = pdram.tile([OWN, E], FP)
            gtw_out = pdram.tile([4 * OWN, E], FP)
            rs_in = [pdram.tile([OWN, D], BF, name=f"rs_in{r}") for r in range(4)]
            rs_out = [pdram.tile([128, D], BF, name=f"rs_out{r}") for r in range(4)]

            # ---------- constants / small weights ----------
            # (off the sync queue so x_sh chunks stream uninterrupted)
            ident = po.tile([128, 128], FP)
            nc.scalar.dma_start(ident[:], dp["ident"][:])

            def load_pcol(name, n, blocks, eng=None):  # [n*128] -> [128, blocks]
                t = po.tile([128, blocks], FP, name=f"{name}_sb")
                (eng or nc.scalar).dma_start(
                    t[:], dp[name].ap().rearrange("(m p) -> p m", p=128))
                return t

            def load_vec1(name, n, eng=None):  # [n] -> [n, 1]
                t = po.tile([n, 1], FP, name=f"{name}_sb")
                (eng or nc.scalar).dma_start(
                    t[:], dp[name].ap().rearrange("(s o) -> s o", o=1))
                return t

            def load_row(name, n, dt_=FP, eng=None):  # [n] -> [1, n]
                t = po.tile([1, n], dt_, name=f"{name}_sb")
                (eng or nc.scalar).dma_start(
                    t[:], dp[name].ap().rearrange("(o s) -> o s", o=1))
                return t

            ones1 = po.tile([1, 128], FR)
            nc.scalar.dma_start(ones1[:], dp["ones1"][:])
            ones1b = po.tile([1, 128], BF)
            nc.scalar.dma_start(ones1b[:], dp["ones1b"][:])

            # persistent activations (live into MoE phase)
            xo = [po.tile([128, D], FP, name=f"xo{t_}", tag=f"xo{t_}") for t_ in range(OTB)]
            xmid = [po.tile([128, D], FP, name=f"xmid{t_}", tag=f"xmid{t_}") for t_ in range(OTB)]
            wv_sb = [po.tile([128, E], FP, name=f"wv{t_}", tag=f"wv{t_}") for t_ in range(OTB)]

            # =======================================================
            # MIXER
            # =======================================================
            with (
                tc.tile_pool(name="mixer", bufs=1) as pm,
                tc.tile_pool(name="mixt", bufs=1) as pt_pool,
            ):
                hT = [pm.tile([128, NH], BF, name=f"hT{kb}", tag=f"hT{kb}") for kb in range(KB)]
                xm = [pm.tile([128, NH], BF, name=f"xm{m}", tag=f"xm{m}") for m in range(MB)]

                # ---- rmsnorm1 + transpose to hT (bf16) ----
                with nc.named_scope("rms1"), tc.tile_pool(name="ps1", bufs=1, space="PSUM") as psA:
                    for tb in range(NH // 128):
                        if tb == 0:
                            xt = pt_pool.tile([128, D], FP, tag="xt", bufs=2)
                        else:
                            xt = xo[tb - 1]
                        nc.sync.dma_start(xt[:], dp["x_sh"][tb * 128:(tb + 1) * 128, :])
                        scr = pt_pool.tile([128, D], FP, tag="scr", bufs=1)
                        sq = pt_pool.tile([128, 1], FP, tag="sq", bufs=2)
                        nc.scalar.activation(scr[:], xt[:], AF.Square, accum_out=sq[:])
                        nr = pt_pool.tile([128, 1], FP, tag="nr", bufs=2)
                        nc.vector.tensor_scalar(nr[:], sq[:], 1.0 / D, 1e-6, ALU.mult, ALU.add)
                        nc.scalar.sqrt(nr[:], nr[:])
                        nc.vector.reciprocal(nr[:], nr[:])
                        h_t = pt_pool.tile([128, D], FP, tag="scr", bufs=1)
                        nc.vector.tensor_scalar(h_t[:], xt[:], nr[:], None, ALU.mult)
                        for kb in range(KB):
                            ptr = psA.tile([128, 128], FP, tag="ptr", bufs=2)
                            nc.tensor.transpose(ptr[:], h_t[:, kb * 128:(kb + 1) * 128], ident[:])
                            nc.vector.tensor_copy(hT[kb][:, tb * 128:(tb + 1) * 128], ptr[:])

                ipb_sb = load_pcol("ipb", 2 * INNER, 32)
                cb_sb = load_pcol("cb", INNER, 16)
                cw_sb = po.tile([128, 16, 3], FP)  # [p, m, k]
                nc.scalar.dma_start(cw_sb[:], dp["cw"].ap().rearrange("(m p) k -> p m k", p=128))

                # ---- in_proj (x_main half) + conv + silu ----
                with nc.named_scope("in_proj"), tc.tile_pool(name="ps2", bufs=1, space="PSUM") as psA:
                    for q in range(4):
                        wq = pt_pool.tile([128, KB, 512], BF, tag="wslab", bufs=2,
                                          name=f"wip{q}")
                        nc.gpsimd.dma_start(
                            wq[:], dp["ipw"].ap()[:, q * 512:(q + 1) * 512]
                            .rearrange("(kb p) n -> p kb n", p=128))
                        for mi in range(4):
                            m = q * 4 + mi
                            xzp = pt_pool.tile([128, NH + 2], BF, tag="xzp", bufs=2)
                            nc.vector.memset(xzp[:, 0:2], 0.0)
                            for n0, nw in ((0, 512), (512, 128)):
                                px = psA.tile([128, 512], FP, tag="px", bufs=2)
                                for kb in range(KB):
                                    nc.tensor.matmul(px[:, 0:nw],
                                                     wq[:, kb, mi * 128:(mi + 1) * 128],
                                                     hT[kb][:, n0:n0 + nw],
                                                     start=(kb == 0), stop=(kb == KB - 1))
                                nc.scalar.activation(xzp[:, 2 + n0:2 + n0 + nw], px[:, 0:nw],
                                                     AF.Identity, bias=ipb_sb[:, m:m + 1])
                            cv = pt_pool.tile([128, NH], BF, tag="cv", bufs=2)
                            nc.vector.tensor_scalar(cv[:], xzp[:, 0:NH], cw_sb[:, m, 0:1],
                                                    None, ALU.mult)
                            nc.vector.scalar_tensor_tensor(cv[:], xzp[:, 1:1 + NH],
                                                           cw_sb[:, m, 1:2], cv[:],
                                                           ALU.mult, ALU.add)
                            nc.vector.scalar_tensor_tensor(cv[:], xzp[:, 2:2 + NH],
                                                           cw_sb[:, m, 2:3], cv[:],
                                                           ALU.mult, ALU.add)
                            sgc = pt_pool.tile([128, NH], BF, tag="sgc", bufs=2)
                            nc.scalar.activation(sgc[:], cv[:], AF.Sigmoid, bias=cb_sb[:, m:m + 1])
                            nc.vector.scalar_tensor_tensor(xm[m][:], cv[:], cb_sb[:, m:m + 1],
                                                           sgc[:], ALU.add, ALU.mult)

                dtb_sb = load_vec1("dtb", S)
                bpb_sb = load_vec1("bpb", S)
                cpb_sb = load_vec1("cpb", S)
                dbw_sb = pm.tile([128, MB, 128], BF, name="dbw_sb")
                nc.scalar.dma_start(dbw_sb[:], dp["dbw"].ap().rearrange("(kb p) s -> p kb s", p=128))
                cpw_sb = pm.tile([128, MB, S], BF, name="cpw_sb")
                nc.scalar.dma_start(cpw_sb[:], dp["cpw"].ap().rearrange("(kb p) s -> p kb s", p=128))

                # ---- dt/B/C projections + scan ----
                with nc.named_scope("scan"), tc.tile_pool(name="ps3", bufs=1, space="PSUM") as psA:
                    dt_t = pt_pool.tile([S, NH], FP, tag="dt")
                    a_t = pt_pool.tile([S, NH], FP, tag="a")
                    b_t = pt_pool.tile([S, NH], FP, tag="b")
                    c_t = pt_pool.tile([S, NH], FP, tag="c")
                    for n0, nw in ((0, 320), (320, 320)):
                        pzdb = psA.tile([128, 320], FP, tag="pzdb", bufs=2)
                        for kb in range(MB):
                            nc.tensor.matmul(pzdb[:, 0:nw], dbw_sb[:, kb, :],
                                             xm[kb][:, n0:n0 + nw],
                                             start=(kb == 0), stop=(kb == MB - 1))
                        nc.scalar.activation(dt_t[:, n0:n0 + nw], pzdb[0:S, 0:nw],
                                             AF.Sigmoid, bias=dtb_sb[:])
                        nc.vector.scalar_tensor_tensor(b_t[:, n0:n0 + nw], pzdb[S:128, 0:nw],
                                                       bpb_sb[:], dt_t[:, n0:n0 + nw],
                                                       ALU.add, ALU.mult)
                        pzc = psA.tile([S, 320], FP, tag="pzc", bufs=2)
                        for kb in range(MB):
                            nc.tensor.matmul(pzc[:, 0:nw], cpw_sb[:, kb, :],
                                             xm[kb][:, n0:n0 + nw],
                                             start=(kb == 0), stop=(kb == MB - 1))
                        nc.scalar.activation(c_t[:, n0:n0 + nw], pzc[:, 0:nw], AF.Identity,
                                             bias=cpb_sb[:])
                    nc.scalar.activation(a_t[:], dt_t[:], AF.Identity, bias=1.0, scale=-1.0)
                    st_t = pt_pool.tile([S, NH], FP, tag="st")
                    nc.vector.tensor_tensor_scan(st_t[:], a_t[:], b_t[:], 0.0,
                                                 ALU.mult, ALU.add)
                    y_t = pt_pool.tile([S, OWN], FP, tag="dt", name="y_t")
                    nc.vector.tensor_mul(y_t[:], c_t[:, HALO:NH], st_t[:, HALO:NH])

                # ---- layernorm over S (transpose - LN - transpose back) ----
                with nc.named_scope("ln"), tc.tile_pool(name="ps4", bufs=1, space="PSUM") as psA:
                    yln = pt_pool.tile([S, OWN], BF, tag="a", name="yln")
                    for i in range(OTB):
                        ptr = psA.tile([128, 128], FP, tag="ptr", bufs=2)
                        nc.tensor.transpose(ptr[:, 0:S], y_t[:, i * 128:(i + 1) * 128],
                                            ident[0:S, 0:S])
                        yT = pt_pool.tile([128, S], FP, tag="yT", bufs=2)
                        nc.vector.tensor_copy(yT[:], ptr[:, 0:S])
                        mu = pt_pool.tile([128, 1], FP, tag="mu", bufs=2)
                        nc.vector.tensor_reduce(mu[:], yT[:], mybir.AxisListType.X, ALU.add)
                        nc.vector.tensor_scalar_mul(mu[:], mu[:], 1.0 / S)
                        xc = pt_pool.tile([128, S], FP, tag="xc", bufs=2)
                        nc.vector.tensor_scalar_sub(xc[:], yT[:], mu[:])
                        scr2 = pt_pool.tile([128, S], FP, tag="scr2", bufs=2)
                        vv = pt_pool.tile([128, 1], FP, tag="vv", bufs=2)
                        nc.scalar.activation(scr2[:], xc[:], AF.Square, accum_out=vv[:])
                        nc.vector.tensor_scalar(vv[:], vv[:], 1.0 / S, 1e-5, ALU.mult, ALU.add)
                        nc.scalar.sqrt(vv[:], vv[:])
                        nc.vector.reciprocal(vv[:], vv[:])
                        nc.vector.tensor_scalar_mul(xc[:], xc[:], vv[:])
                        ptr2 = psA.tile([128, 128], FP, tag="ptr2", bufs=2)
                        nc.tensor.transpose(ptr2[0:S, :], xc[:], ident[:])
                        nc.vector.tensor_copy(yln[:, i * 128:(i + 1) * 128], ptr2[0:S, :])

                s2ib_sb = load_pcol("s2ib", INNER, 16)
                Dp_sb = load_pcol("Dp", INNER, 16)
                s2iw_sb = pm.tile([S, INNER], BF, name="s2iw_sb")
                nc.scalar.dma_start(s2iw_sb[:], dp["s2iw"][:])

                # ---- s2i + gate sigmoid + pre_out assembly ----
                with nc.named_scope("premix"), tc.tile_pool(name="ps5", bufs=1, space="PSUM") as psA:
                    pre = []
                    for m in range(MB):
                        q, mi = divmod(m, 4)
                        if mi == 0:
                            wq = pt_pool.tile([128, KB, 512], BF, tag="wslab", bufs=2,
                                              name=f"wipg{q}")
                            nc.gpsimd.dma_start(
                                wq[:], dp["ipw"].ap()[:, 2048 + q * 512:2048 + (q + 1) * 512]
                                .rearrange("(kb p) n -> p kb n", p=128))
                        ps = psA.tile([128, 512], FP, tag="ps", bufs=2)
                        nc.tensor.matmul(ps[:], s2iw_sb[:, m * 128:(m + 1) * 128], yln[:],
                                         start=True, stop=True)
                        pg = psA.tile([128, 512], FP, tag="pg", bufs=2)
                        for kb in range(KB):
                            nc.tensor.matmul(pg[:], wq[:, kb, mi * 128:(mi + 1) * 128],
                                             hT[kb][:, HALO:NH],
                                             start=(kb == 0), stop=(kb == KB - 1))
                        sg = pt_pool.tile([128, OWN], FP, tag="sg", bufs=2)
                        nc.scalar.activation(sg[:], pg[:], AF.Sigmoid,
                                             bias=ipb_sb[:, MB + m:MB + m + 1])
                        tmp = pt_pool.tile([128, OWN], FP, tag="tmp", bufs=2)
                        nc.vector.tensor_scalar(tmp[:], xm[m][:, HALO:NH],
                                                Dp_sb[:, m:m + 1], None, ALU.mult)
                        nc.vector.scalar_tensor_tensor(tmp[:], ps[:], s2ib_sb[:, m:m + 1],
                                                       tmp[:], ALU.add, ALU.add)
                        pre_m = pm.tile([128, OWN], BF, tag=f"xm{m}", name=f"pre{m}")
                        nc.vector.tensor_mul(pre_m[:], tmp[:], sg[:])
                        pre.append(pre_m)

                obb_sb = load_row("ob", D, BF)
                gw_sb = po.tile([128, KB, E], FP)  # [p, kb, e]
                nc.scalar.dma_start(gw_sb[:], dp["gw"].ap().rearrange("(kb p) e -> p kb e", p=128))
                gb_sb = load_row("gb", E, FR)
                owsb = [pm.tile([128, D], BF, name=f"owsb{kb}", tag=f"owsb{kb}")
                        for kb in range(MB)]
                for kb in range(MB):
                    nc.scalar.dma_start(owsb[kb][:], dp["ow"][kb * 128:(kb + 1) * 128, :])

                # ---- tb-major: out projection + rms2 + h2T + AG per tb ----
                with nc.named_scope("outgate"), tc.tile_pool(name="ps7", bufs=1, space="PSUM") as psA:
                    for tb in range(OTB):
                        potn = [psA.tile([128, 512], FP, tag=f"pon{nb}", bufs=2,
                                         name=f"pon{nb}_{tb}") for nb in range(2)]
                        for kb in range(MB):
                            for nb in range(2):
                                nc.tensor.matmul(potn[nb][:],
                                                 pre[kb][:, tb * 128:(tb + 1) * 128],
                                                 owsb[kb][:, nb * 512:(nb + 1) * 512],
                                                 start=(kb == 0), stop=False)
                        for nb in range(2):
                            nc.tensor.matmul(potn[nb][:], ones1b[:],
                                             obb_sb[:, nb * 512:(nb + 1) * 512],
                                             start=False, stop=True)
                            nc.vector.tensor_add(xmid[tb][:, nb * 512:(nb + 1) * 512],
                                                 potn[nb][:],
                                                 xo[tb][:, nb * 512:(nb + 1) * 512])
                        scr = pt_pool.tile([128, D], FP, tag="scr", bufs=1)
                        sq = pt_pool.tile([128, 1], FP, tag="sq", bufs=2)
                        nc.scalar.activation(scr[:], xmid[tb][:], AF.Square, accum_out=sq[:])
                        nr = pt_pool.tile([128, 1], FP, tag="nr", bufs=2)
                        nc.vector.tensor_scalar(nr[:], sq[:], 1.0 / D, 1e-6, ALU.mult, ALU.add)
                        nc.scalar.sqrt(nr[:], nr[:])
                        nc.vector.reciprocal(nr[:], nr[:])
                        h2 = pt_pool.tile([128, D], FP, tag="xt", bufs=2, name="h2")
                        nc.vector.tensor_scalar(h2[:], xmid[tb][:], nr[:], None, ALU.mult)
                        pl = psA.tile([128, E], FP, tag="pl", bufs=2)
                        for kb in range(KB):
                            ptr = psA.tile([128, 128], FP, tag="ptr", bufs=2)
                            nc.tensor.transpose(ptr[:], h2[:, kb * 128:(kb + 1) * 128], ident[:])
                            h2T_t = pt_pool.tile([128, 128], FP, tag="h2T", bufs=2)
                            nc.vector.tensor_copy(h2T_t[:], ptr[:])
                            nc.tensor.matmul(pl[:], h2T_t[:], gw_sb[:, kb, :],
                                             start=(kb == 0), stop=False)
                        nc.tensor.matmul(pl[:], ones1[:], gb_sb[:], start=False, stop=True)
                        # top-2-of-4 gating
                        m1 = pt_pool.tile([128, 1], FP, tag="m1", bufs=2)
                        nc.vector.tensor_reduce(m1[:], pl[:], mybir.AxisListType.X, ALU.max)
                        eq1 = pt_pool.tile([128, E], FP, tag="eq1", bufs=2)
                        nc.vector.tensor_scalar(eq1[:], pl[:], m1[:], None, ALU.is_equal)
                        msk = pt_pool.tile([128, E], FP, tag="msk", bufs=2)
                        nc.vector.scalar_tensor_tensor(msk[:], eq1[:], -1e30, pl[:],
                                                       ALU.mult, ALU.add)
                        m2 = pt_pool.tile([128, 1], FP, tag="m2", bufs=2)
                        nc.vector.tensor_reduce(m2[:], msk[:], mybir.AxisListType.X, ALU.max)
                        eq2 = pt_pool.tile([128, E], FP, tag="eq2", bufs=2)
                        nc.vector.tensor_scalar(eq2[:], msk[:], m2[:], None, ALU.is_equal)
                        dd = pt_pool.tile([128, 1], FP, tag="dd", bufs=2)
                        nc.vector.tensor_sub(dd[:], m2[:], m1[:])
                        p2 = pt_pool.tile([128, 1], FP, tag="p2", bufs=2)
                        nc.scalar.activation(p2[:], dd[:], AF.Sigmoid)
                        p1b = pt_pool.tile([128, 1], FP, tag="p1b", bufs=2)
                        nc.scalar.activation(p1b[:], p2[:], AF.Identity, bias=1.0, scale=-1.0)
                        nc.vector.tensor_scalar(wv_sb[tb][:], eq1[:], p1b[:], None, ALU.mult)
                        nc.vector.scalar_tensor_tensor(wv_sb[tb][:], eq2[:], p2[:], wv_sb[tb][:],
                                                       ALU.mult, ALU.add)
                        nc.sync.dma_start(gtw_in[tb * 128:(tb + 1) * 128, :], wv_sb[tb][:])
                        h2b = pt_pool.tile([128, D], BF, tag="h2b", bufs=2)
                        nc.vector.tensor_copy(h2b[:], h2[:])
                        gin = gth_in_a if tb < 2 else gth_in_b
                        nc.sync.dma_start(gin[(tb % 2) * 128:(tb % 2 + 1) * 128, :], h2b[:])
                        if tb == 1:
                            nc.gpsimd.collective_compute(
                                "AllGather", ALU.bypass, replica_groups=rg,
                                ins=[gth_in_a.opt()],
                                outs=[gth_all[0:1024, :].opt()])
                    with nc.named_scope("gather"):
                        nc.gpsimd.collective_compute(
                            "AllGather", ALU.bypass, replica_groups=rg,
                            ins=[gtw_in.opt()], outs=[gtw_out.opt()])
                        nc.gpsimd.collective_compute(
                            "AllGather", ALU.bypass, replica_groups=rg,
                            ins=[gth_in_b.opt()],
                            outs=[gth_all[1024:2048, :].opt()])

            # =======================================================
            # MoE (full expert per core, token-half group of 4)
            # =======================================================
            with (
                tc.tile_pool(name="moe", bufs=1) as pq,
                tc.tile_pool(name="psC", bufs=1, space="PSUM") as psC,
            ):
                HB = 2 * HH // 128  # 32 hid blocks
                with nc.named_scope("moe_w"):
                    # ew1 on the scalar queue (first), ew2 resident on sync:
                    # round-0 matmuls only need ew1 kb=0 so compute starts while
                    # the rest streams in.
                    ew1_sb = [pq.tile([128, 2 * HH], BF, name=f"ew1_{kb}", tag=f"ew1_{kb}")
                              for kb in range(KB)]
                    for kb in range(KB):
                        nc.scalar.dma_start(ew1_sb[kb][:], dp["ew1"][kb * 128:(kb + 1) * 128, :])
                    ew2R = [pq.tile([128, HB, 512], F8, name=f"ew2R{nb}", tag=f"ew2R{nb}")
                            for nb in range(2)]
                    for nb in range(2):
                        nc.sync.dma_start(
                            ew2R[nb][:], dp["ew2"].ap()[:, nb * 512:(nb + 1) * 512]
                            .rearrange("(hb p) d -> p hb d", p=128))
                esel = po.tile([128, E], FP)
                nc.scalar.dma_start(esel[:], dp["esel"][:])
                rmask = po.tile([128, 4], FP)
                nc.scalar.dma_start(rmask[:], dp["rmask"][:])
                eb1_sb = load_pcol("eb1", 2 * HH, 32)
                eb2h_sb = load_row("eb2h", D, BF)

                with nc.named_scope("moe"):
                    NP = 384          # padded selected-token count per quarter
                    NPB = NP // 128   # 3 compact token blocks
                    iotaG = po.tile([128, 16], FP)
                    nc.scalar.dma_start(iotaG[:], dp["iotaG"][:])
                    iotaL = po.tile([128, 4], FP)
                    nc.scalar.dma_start(iotaL[:], dp["iotaL"][:])
                    ltm = po.tile([128, 128], FP)
                    nc.scalar.dma_start(ltm[:], dp["ltmat"][:])
                    pre12 = po.tile([128, 3, 4], FP)
                    nc.scalar.dma_start(
                        pre12[:], dp["pre12"].ap().rearrange("p (b c) -> p b c", b=3))
                    ones4 = pq.tile([128, 4], FP)
                    nc.vector.memset(ones4[:], 1.0)

                    # -------- per-round index build (scatter-compact) --------
                    wcomp_r, loci_r, idxi_r = [], [], []
                    for r in range(4):
                        nc.sync.dma_start(
                            pay[r][:, :].rearrange("(b p) c -> p b c", p=128), pre12[:])
                        wvr = pq.tile([128, OTB, E], FP, tag="wvr", bufs=2)
                        nc.sync.dma_start(
                            wvr[:], gtw_out[r * OWN:(r + 1) * OWN, :]
                            .rearrange("(tb p) e -> p tb e", p=128))
                        wsall = pq.tile([128, 4], FP, tag="wsall", bufs=2)
                        for tb in range(OTB):
                            wm_t = pq.tile([128, E], FP, tag="wm", bufs=2)
                            nc.vector.tensor_mul(wm_t[:], wvr[:, tb, :], esel[:])
                            nc.vector.tensor_reduce(wsall[:, tb:tb + 1], wm_t[:],
                                                    mybir.AxisListType.X, ALU.add)
                        msk = pq.tile([128, 4], FP, tag="msk", bufs=2)
                        nc.vector.tensor_scalar(msk[:], wsall[:], 0.0, None, ALU.is_gt)
                        csum = pq.tile([128, 4], FP, tag="csum", bufs=2)
                        nc.vector.tensor_tensor_scan(csum[:], ones4[:], msk[:], 0.0,
                                                     ALU.mult, ALU.add)
                        pbase = psC.tile([128, 1], FP, tag="ph", bufs=2, name="pbase")
                        nc.tensor.matmul(pbase[:], ltm[:], csum[:, 3:4],
                                         start=True, stop=True)
                        pos = pq.tile([128, 4], FP, tag="pos", bufs=2)
                        nc.vector.tensor_sub(pos[:], csum[:], msk[:])
                        nc.vector.tensor_scalar(pos[:], pos[:], pbase[:], None, ALU.add)
                        dpos = pq.tile([128, 4], FP, tag="dpos", bufs=2)
                        nc.vector.tensor_scalar(dpos[:], pos[:], -4096.0, None, ALU.add)
                        nc.vector.tensor_mul(dpos[:], dpos[:], msk[:])
                        nc.vector.tensor_scalar(dpos[:], dpos[:], 4096.0, None, ALU.add)
                        posi = pq.tile([128, 4], mybir.dt.int32, tag="posi", bufs=2)
                        nc.vector.tensor_copy(posi[:], dpos[:])
                        for tb in range(OTB):
                            payt = pq.tile([128, 4], FP, tag="payt", bufs=2)
                            nc.vector.tensor_copy(payt[:, 0:1], iotaG[:, r * 4 + tb:r * 4 + tb + 1])
                            nc.vector.tensor_copy(payt[:, 1:2], iotaL[:, tb:tb + 1])
                            nc.vector.tensor_copy(payt[:, 2:3], wsall[:, tb:tb + 1])
                            nc.vector.memset(payt[:, 3:4], 0.0)
                            nc.gpsimd.indirect_dma_start(
                                out=pay[r][:], out_offset=bass.IndirectOffsetOnAxis(
                                    ap=posi[:, tb:tb + 1], axis=0),
                                in_=payt[:], in_offset=None,
                                bounds_check=NP - 1, oob_is_err=False)
                        # readbacks
                        idxf = pq.tile([128, NP // 16], FP, tag="idxf", bufs=2)
                        for g in range(8):
                            nc.sync.dma_start(
                                idxf[g * 16:(g + 1) * 16, :],
                                pay[r][:, 0:1].rearrange("(c p) o -> p (c o)", p=16))
                        idxi = pq.tile([128, NP // 16], mybir.dt.int16, tag=f"idxi{r}",
                                       bufs=1, name=f"idxi{r}")
                        nc.vector.tensor_copy(idxi[:], idxf[:])
                        wcf = pq.tile([128, NPB], FP, tag=f"wcf{r}", bufs=1, name=f"wcf{r}")
                        nc.sync.dma_start(
                            wcf[:], pay[r][:, 2:3].rearrange("(b p) o -> p (b o)", p=128))
                        locf = pq.tile([128, NPB], FP, tag="locf", bufs=2)
                        nc.sync.dma_start(
                            locf[:], pay[r][:, 1:2].rearrange("(b p) o -> p (b o)", p=128))
                        loci = pq.tile([128, NPB], mybir.dt.int32, tag=f"loci{r}",
                                       bufs=1, name=f"loci{r}")
                        nc.vector.tensor_copy(loci[:], locf[:])
                        if r == 0:
                            h2g0 = pq.tile([128, KB, NP], BF, tag="h2g0", bufs=1,
                                           name="h2g0")
                            nc.gpsimd.dma_gather(
                                h2g0[:], gth_all[:], idxi[:], NP, NP,
                                elem_size=D, transpose=True)
                        idxi_r.append(idxi)
                        wcomp_r.append(wcf)
                        loci_r.append(loci)

                    # -------- residual prefill for all rounds (bf16) --------
                    # own-expert quarter carries xmid, others zero; done here so
                    # the compute rounds never wait on it.
                    for r in range(4):
                        for tb in range(OTB):
                            xmr = pq.tile([128, D], BF, tag="xmr", bufs=2)
                            nc.vector.tensor_scalar(xmr[:], xmid[tb][:], rmask[:, r:r + 1],
                                                    None, ALU.mult)
                            nc.scalar.dma_start(rs_in[r][tb * 128:(tb + 1) * 128, :], xmr[:])

                    # -------- per-round compute on compacted tokens --------
                    h2g_t = [h2g0, None, None, None]
                    for r in range(4):
                        if r < 3:  # prefetch next round's tokens during this one
                            nxt = pq.tile([128, KB, NP], BF, tag="h2g", bufs=2,
                                          name=f"h2g{r + 1}")
                            nc.gpsimd.dma_gather(
                                nxt[:], gth_all[:], idxi_r[r + 1][:], NP, NP,
                                elem_size=D, transpose=True)
                            h2g_t[r + 1] = nxt
                        h2g = h2g_t[r]
                        hidp = [pq.tile([128, 2, NP], F8, tag=f"hidp{p}", bufs=1,
                                        name=f"hidp{p}") for p in range(HB // 2)]
                        for h in range(HB):
                            ph = psC.tile([128, NP], FP, tag="ph", bufs=2)
                            for kb in range(KB):
                                nc.tensor.matmul(ph[:], ew1_sb[kb][:, h * 128:(h + 1) * 128],
                                                 h2g[:, kb, :], start=(kb == 0),
                                                 stop=(kb == KB - 1))
                            nc.scalar.activation(hidp[h // 2][:, h % 2, :], ph[:],
                                                 AF.Gelu, bias=eb1_sb[:, h:h + 1])
                        peo = [[psC.tile([128, 512], FP, tag=f"peo{b}n{nb}", bufs=1,
                                         name=f"peo{b}n{nb}") for nb in range(2)]
                               for b in range(NPB)]
                        for nb in range(2):
                            for p in range(16):
                                for b in range(NPB):
                                    nc.tensor.matmul(
                                        peo[b][nb][:],
                                        hidp[p][:, :, b * 128:(b + 1) * 128],
                                        ew2R[nb][:, 2 * p:2 * p + 2, :],
                                        start=(p == 0), stop=False, perf_mode=DR)
                            for b in range(NPB):
                                nc.tensor.matmul(peo[b][nb][:], ones1b[:],
                                                 eb2h_sb[:, nb * 512:(nb + 1) * 512],
                                                 start=False, stop=True)
                        for b in range(NPB):
                            wf = pq.tile([128, D], BF, tag="wf", bufs=3)
                            for nb in range(2):
                                nc.vector.tensor_scalar(wf[:, nb * 512:(nb + 1) * 512],
                                                        peo[b][nb][:],
                                                        wcomp_r[r][:, b:b + 1],
                                                        None, ALU.mult)
                            nc.gpsimd.indirect_dma_start(
                                out=rs_in[r][:], out_offset=bass.IndirectOffsetOnAxis(
                                    ap=loci_r[r][:, b:b + 1], axis=0),
                                in_=wf[:], in_offset=None,
                                bounds_check=OWN - 1, oob_is_err=False,
                                compute_op=ALU.add)
                        nc.gpsimd.collective_compute(
                            "ReduceScatter", ALU.add, replica_groups=rg,
                            ins=[rs_in[r].opt()], outs=[rs_out[r].opt()])
                        nc.sync.dma_start(out_d[r * 128:(r + 1) * 128, :], rs_out[r][:])

    nc.compile()
    return nc


def host_prep(inputs):
    """Build the 8 per-core input maps from full inputs."""
    import ml_dtypes
    f32 = np.float32
    bf16 = ml_dtypes.bfloat16
    x = np.ascontiguousarray(np.asarray(inputs["x"], f32).reshape(B * T, D))
    n1 = np.asarray(inputs["norm1_w"], f32)
    n2 = np.asarray(inputs["norm2_w"], f32)
    ipw = np.ascontiguousarray(
        (np.asarray(inputs["in_proj_w"], f32) * n1[:, None]).astype(bf16))
    gw = np.ascontiguousarray(np.asarray(inputs["gate_w"], f32) * n2[:, None])
    import ml_dtypes as mld
    ew1f = np.asarray(inputs["e_w1"], f32) * n2[None, :, None]
    ew1b = ew1f.astype(bf16)
    ew2f = np.asarray(inputs["e_w2"], f32)
    # per-expert power-of-2 scale into fp8 e4m3 range (max ~240)
    s2 = np.array([2.0 ** np.floor(np.log2(192.0 / max(np.abs(ew2f[e]).max(), 1e-9)))
                   for e in range(E)], f32)
    ew2q = np.stack([(ew2f[e] * s2[e]).astype(mld.float8_e4m3) for e in range(E)])
    dbw = np.ascontiguousarray(np.concatenate(
        [np.asarray(inputs["dt_w"], f32), np.asarray(inputs["bp_w"], f32)],
        axis=1).astype(bf16))
    ident = np.eye(128, dtype=f32)
    ones1 = np.ones((1, 128), f32)
    pp = np.arange(128, dtype=f32)
    iotaG = np.empty((128, 16), f32)
    for r_ in range(4):
        for tb_ in range(4):
            iotaG[:, r_ * 4 + tb_] = (tb_ // 2) * 1024 + r_ * 256 + (tb_ % 2) * 128 + pp
    iotaL = np.empty((128, 4), f32)
    for tb_ in range(4):
        iotaL[:, tb_] = tb_ * 128 + pp
    ltmat = (pp[:, None] < pp[None, :]).astype(f32)
    pre12 = np.tile(np.array([0.0, 4095.0, 0.0, 0.0], f32), 3)[None, :].repeat(128, 0)
    shared = {
        "ipw": ipw, "ipb": np.asarray(inputs["in_proj_b"], f32),
        "cw": np.ascontiguousarray(np.asarray(inputs["conv_w"], f32)[:, 0, :]),
        "cb": np.asarray(inputs["conv_b"], f32),
        "dbw": dbw,
        "dtb": np.asarray(inputs["dt_b"], f32), "bpb": np.asarray(inputs["bp_b"], f32),
        "cpw": np.asarray(inputs["cp_w"], f32).astype(bf16),
        "cpb": np.asarray(inputs["cp_b"], f32),
        "s2iw": np.asarray(inputs["s2i_w"], f32).astype(bf16),
        "s2ib": np.asarray(inputs["s2i_b"], f32),
        "Dp": np.asarray(inputs["D_param"], f32),
        "ow": np.asarray(inputs["out_w"], f32).astype(bf16),
        "ob": np.asarray(inputs["out_b"], f32).astype(bf16),
        "gw": gw, "gb": np.asarray(inputs["gate_b"], f32),
        "ident": ident, "ones1": ones1, "ones1b": ones1.astype(bf16),
        "iotaG": iotaG, "iotaL": iotaL, "ltmat": np.ascontiguousarray(ltmat),
        "pre12": np.ascontiguousarray(pre12),
    }
    eb1 = np.asarray(inputs["e_b1"], f32)
    eb2 = np.asarray(inputs["e_b2"], f32)
    in_maps = []
    for c in range(N_CORES):
        e, th = c // 2, c % 2
        g0 = th * (B * T // 2) + e * OWN
        if e == 0:
            x_sh = np.concatenate([np.zeros((HALO, D), f32), x[g0:g0 + OWN]])
        else:
            x_sh = x[g0 - HALO:g0 + OWN]
        m = dict(shared)
        m["x_sh"] = np.ascontiguousarray(x_sh)
        m["ew1"] = np.ascontiguousarray(ew1b[e])
        m["eb1"] = np.ascontiguousarray(eb1[e])
        m["ew2"] = np.ascontiguousarray(ew2q[e])
        m["eb2h"] = np.ascontiguousarray((eb2[e] * s2[e]).astype(bf16))
        esel = np.zeros((128, E), f32)
        esel[:, e] = 1.0 / s2[e]  # dequant of fp8-scaled ew2 folded into combine weight
        m["esel"] = esel
        rmask = np.zeros((128, 4), f32)
        rmask[:, e] = 1.0
        m["rmask"] = rmask
        in_maps.append(m)
    return in_maps


def unshard_out(results):
    """results: list of 8 dicts with 'out' [OWN, D]; rows r*128+i of core c
    hold global token (c%2)*2048 + r*512 + (c//2)*128 + i."""
    full = np.empty((B * T, D), np.float32)
    for c in range(N_CORES):
        e, th = c // 2, c % 2
        oc = results[c]["out"]
        for r in range(4):
            full[th * 2048 + r * OWN + e * 128: th * 2048 + r * OWN + (e + 1) * 128] = \
                oc[r * 128:(r + 1) * 128]
    return full.reshape(B, T, D)


_NC_CACHE = {}


def _get_nc():
    if "nc" not in _NC_CACHE:
        _NC_CACHE["nc"] = build(debug_outputs=False)
    return _NC_CACHE["nc"]


def kernel(**inputs) -> np.ndarray:
    """Full-input entry point: shards across 8 NeuronCores, runs the Bass
    kernel SPMD, reassembles the full [2, 2048, 1024] output."""
    import sys, types
    try:  # NTFF profile hook shim (missing antenv.axon_hooks in this image)
        import antenv.axon_hooks  # noqa: F401
    except ImportError:
        try:
            import antenv
            from trn_agent_boot.trn_boot import _ntff_profile_via_ctypes
            mod = types.ModuleType("antenv.axon_hooks")
            try:
                _hook = _ntff_profile_via_ctypes("/opt/axon/libaxon_pjrt.so")
            except Exception:
                _hook = None
            mod.get_axon_ntff_profile_hook = lambda: _hook
            mod.set_axon_ntff_profile_hook = lambda h: None
            sys.modules["antenv.axon_hooks"] = mod
            antenv.axon_hooks = mod
        except Exception:
            pass
    from concourse.bass_utils import run_bass_kernel_spmd

    nc = _get_nc()
    in_maps = host_prep(inputs)
    res = run_bass_kernel_spmd(nc, in_maps, core_ids=list(range(N_CORES)))
    out = unshard_out(res.results)
    return out.astype(np.float32)



# revision 38
# speedup vs baseline: 1.0944x; 1.0330x over previous
"""Bass kernel builder for nn_MixtureOfMambaBlock — 8-core SPMD.

Sharding: tokens 8-way (512/core + 128 halo for conv+scan warmup); mixer fully
local per core (weights replicated, bf16 matmuls; gate-logit path kept f32).
Post-mixer h2 all-gathered (bf16), MoE expert-parallel (one expert per core,
dense over the 2048-token half), weighted partials reduce-scattered back (bf16).

Scheduling: ew2 weights SBUF-resident across rounds; next round's token gather
prefetched during the current round; residual prefills + index build front-
loaded; DMA load traffic spread across sync/scalar/vector queues to keep the
PE warm (HAM) and the round critical path on matmuls only.
"""
import numpy as np
import concourse.bass as bass
import concourse.bacc as bacc
import concourse.mybir as mybir
import concourse.tile as tile

FP = mybir.dt.float32
FR = mybir.dt.float32r
BF = mybir.dt.bfloat16
F8 = mybir.dt.float8e4
DR = mybir.MatmulPerfMode.DoubleRow
AF = mybir.ActivationFunctionType
ALU = mybir.AluOpType

B, T, D = 2, 2048, 1024
S, INNER = 64, 2048
E, HH = 4, 2048          # experts, hid-half width
OWN, HALO = 512, 128
NH = OWN + HALO          # 640
KB = D // 128            # 8  d-blocks
MB = INNER // 128        # 16 inner-blocks
OTB = OWN // 128         # 4  own-token blocks
N_CORES = 8
DW = D + 128             # h2 AllGather row width (wv packed at 1024:1028; 256B-aligned)

INPUT_SPECS = {
    "x_sh": ([NH, D], FP),
    "ipw": ([D, 2 * INNER], BF), "ipb": ([2 * INNER], FP),
    "cw": ([INNER, 3], FP), "cb": ([INNER], FP),
    "dbw": ([INNER, 128], BF),  # dt_w || bp_w stacked on output dim
    "dtb": ([S], FP), "bpb": ([S], FP),
    "cpw": ([INNER, S], BF), "cpb": ([S], FP),
    "s2iw": ([S, INNER], BF), "s2ib": ([INNER], FP),
    "Dp": ([INNER], FP),
    "ow": ([INNER, D], BF), "ob": ([D], BF),
    "gw": ([D, E], FP), "gb": ([E], FR),
    "ew1": ([D, 2 * HH], BF), "eb1": ([2 * HH], FP),
    "ew2": ([2 * HH, D], F8), "eb2h": ([D], BF),
    "esel": ([128, E], FP),
    "rmask": ([128, 4], FP),
    "ident": ([128, 128], FP),
    "iotaG": ([128, 16], FP),
    "iotaL": ([128, 4], FP),
    "ltmat": ([128, 128], FP),
    "pre12": ([128, 12], FP),
    "ones1": ([1, 128], FR),
    "ones1b": ([1, 128], BF),
}


def build(debug_outputs=False):
    nc = bacc.Bacc("TRN2", target_bir_lowering=False, debug=False,
                   num_devices=N_CORES)
    dp = {}
    for name, (shape, dt) in INPUT_SPECS.items():
        dp[name] = nc.dram_tensor(name, shape, dt, kind="ExternalInput")
    out_d = nc.dram_tensor("out", [OWN, D], BF, kind="ExternalOutput")

    rg = [[0, 2, 4, 6], [1, 3, 5, 7]]

    with tile.TileContext(nc) as tc:
        with (
            tc.tile_pool(name="outer", bufs=1) as po,
            tc.tile_pool(name="dram", bufs=1, space="DRAM") as pdram,
        ):
            # ---------- DRAM bounce buffers for collectives ----------
            # h2 rows widened to DW: cols 1024:1028 carry the token's gate
            # weights so no separate gtw AllGather is needed.
            gth_in_a = pdram.tile([256, DW], BF)
            gth_in_b = pdram.tile([256, DW], BF)
            gth_all = pdram.tile([4 * OWN, DW], BF)
            pay = [pdram.tile([384, 4], FP, name=f"pay{r}") for r in range(4)]
            rs_in = [pdram.tile([OWN, D], BF, name=f"rs_in{r}") for r in range(4)]
            rs_out = [pdram.tile([128, D], BF, name=f"rs_out{r}") for r in range(4)]

            # ---------- constants / small weights ----------
            # (off the sync queue so x_sh chunks stream uninterrupted)
            ident = po.tile([128, 128], FP)
            nc.scalar.dma_start(ident[:], dp["ident"][:])

            def load_pcol(name, n, blocks, eng=None):  # [n*128] -> [128, blocks]
                t = po.tile([128, blocks], FP, name=f"{name}_sb")
                (eng or nc.scalar).dma_start(
                    t[:], dp[name].ap().rearrange("(m p) -> p m", p=128))
                return t

            def load_vec1(name, n, eng=None):  # [n] -> [n, 1]
                t = po.tile([n, 1], FP, name=f"{name}_sb")
                (eng or nc.scalar).dma_start(
                    t[:], dp[name].ap().rearrange("(s o) -> s o", o=1))
                return t

            def load_row(name, n, dt_=FP, eng=None):  # [n] -> [1, n]
                t = po.tile([1, n], dt_, name=f"{name}_sb")
                (eng or nc.scalar).dma_start(
                    t[:], dp[name].ap().rearrange("(o s) -> o s", o=1))
                return t

            ones1 = po.tile([1, 128], FR)
            nc.scalar.dma_start(ones1[:], dp["ones1"][:])
            ones1b = po.tile([1, 128], BF)
            nc.scalar.dma_start(ones1b[:], dp["ones1b"][:])

            # MoE index-build constants, loaded up front so the build phase
            # never waits behind bulk weight DMAs.
            esel = po.tile([128, E], FP)
            nc.scalar.dma_start(esel[:], dp["esel"][:])
            rmask = po.tile([128, 4], FP)
            nc.scalar.dma_start(rmask[:], dp["rmask"][:])
            iotaG = po.tile([128, 16], FP)
            nc.scalar.dma_start(iotaG[:], dp["iotaG"][:])
            iotaL = po.tile([128, 4], FP)
            nc.scalar.dma_start(iotaL[:], dp["iotaL"][:])
            ltm = po.tile([128, 128], FP)
            nc.scalar.dma_start(ltm[:], dp["ltmat"][:])
            pre12 = po.tile([128, 3, 4], FP)
            nc.scalar.dma_start(
                pre12[:], dp["pre12"].ap().rearrange("p (b c) -> p b c", b=3))
            eb1_sb = load_pcol("eb1", 2 * HH, 32)
            eb2h_sb = load_row("eb2h", D, BF)
            for r in range(4):
                nc.scalar.dma_start(
                    pay[r][:, :].rearrange("(b p) c -> p b c", p=128), pre12[:])

            # persistent activations (live into MoE phase)
            xo = [po.tile([128, D], FP, name=f"xo{t_}", tag=f"xo{t_}") for t_ in range(OTB)]
            xmid = [po.tile([128, D], FP, name=f"xmid{t_}", tag=f"xmid{t_}") for t_ in range(OTB)]
            wv_sb = [po.tile([128, E], FP, name=f"wv{t_}", tag=f"wv{t_}") for t_ in range(OTB)]

            # =======================================================
            # MIXER
            # =======================================================
            with (
                tc.tile_pool(name="mixer", bufs=1) as pm,
                tc.tile_pool(name="mixt", bufs=1) as pt_pool,
            ):
                hT = [pm.tile([128, NH], BF, name=f"hT{kb}", tag=f"hT{kb}") for kb in range(KB)]
                xm = [pm.tile([128, NH], BF, name=f"xm{m}", tag=f"xm{m}") for m in range(MB)]

                # ---- rmsnorm1 + transpose to hT (bf16) ----
                # batched: all DMAs (2 queues) + all squares first, one norm
                # chain, then per-chunk scale+transpose — avoids per-chunk
                # cross-engine ping-pong.
                with nc.named_scope("rms1"), tc.tile_pool(name="ps1", bufs=1, space="PSUM") as psA:
                    NTB = NH // 128
                    xts = []
                    for tb in range(NTB):
                        if tb == 0:
                            xt = pt_pool.tile([128, D], FP, tag="xt", bufs=2, name="xt")
                        else:
                            xt = xo[tb - 1]
                        eng = nc.sync if tb % 2 == 0 else nc.scalar
                        eng.dma_start(xt[:], dp["x_sh"][tb * 128:(tb + 1) * 128, :])
                        xts.append(xt)
                    sqa = pt_pool.tile([128, NTB], FP, tag="sq", bufs=1)
                    for tb in range(NTB):
                        scr = pt_pool.tile([128, D], FP, tag="scr", bufs=2)
                        nc.scalar.activation(scr[:], xts[tb][:], AF.Square,
                                             accum_out=sqa[:, tb:tb + 1])
                    nra = pt_pool.tile([128, NTB], FP, tag="nr", bufs=1)
                    nc.vector.tensor_scalar(nra[:], sqa[:], 1.0 / D, 1e-6, ALU.mult, ALU.add)
                    nc.scalar.sqrt(nra[:], nra[:])
                    nc.vector.reciprocal(nra[:], nra[:])
                    for tb in range(NTB):
                        h_t = pt_pool.tile([128, D], FP, tag="scr", bufs=2)
                        nc.vector.tensor_scalar(h_t[:], xts[tb][:], nra[:, tb:tb + 1],
                                                None, ALU.mult)
                        for kb in range(KB):
                            ptr = psA.tile([128, 128], FP, tag="ptr", bufs=2)
                            nc.tensor.transpose(ptr[:], h_t[:, kb * 128:(kb + 1) * 128], ident[:])
                            nc.vector.tensor_copy(hT[kb][:, tb * 128:(tb + 1) * 128], ptr[:])

                ipb_sb = load_pcol("ipb", 2 * INNER, 32)
                cb_sb = load_pcol("cb", INNER, 16)
                cw_sb = po.tile([128, 16, 3], FP)  # [p, m, k]
                nc.scalar.dma_start(cw_sb[:], dp["cw"].ap().rearrange("(m p) k -> p m k", p=128))

                # ---- in_proj (x_main half) + conv + silu ----
                with nc.named_scope("in_proj"), tc.tile_pool(name="ps2", bufs=1, space="PSUM") as psA:
                    for q in range(4):
                        wq = pt_pool.tile([128, KB, 512], BF, tag="wslab", bufs=2,
                                          name=f"wip{q}")
                        nc.gpsimd.dma_start(
                            wq[:], dp["ipw"].ap()[:, q * 512:(q + 1) * 512]
                            .rearrange("(kb p) n -> p kb n", p=128))
                        for mi in range(4):
                            m = q * 4 + mi
                            xzp = pt_pool.tile([128, NH + 2], BF, tag="xzp", bufs=2)
                            nc.vector.memset(xzp[:, 0:2], 0.0)
                            for n0, nw in ((0, 512), (512, 128)):
                                px = psA.tile([128, 512], FP, tag="px", bufs=2)
                                for kb in range(KB):
                                    nc.tensor.matmul(px[:, 0:nw],
                                                     wq[:, kb, mi * 128:(mi + 1) * 128],
                                                     hT[kb][:, n0:n0 + nw],
                                                     start=(kb == 0), stop=(kb == KB - 1))
                                nc.scalar.activation(xzp[:, 2 + n0:2 + n0 + nw], px[:, 0:nw],
                                                     AF.Identity, bias=ipb_sb[:, m:m + 1])
                            cv = pt_pool.tile([128, NH], BF, tag="cv", bufs=2)
                            nc.vector.tensor_scalar(cv[:], xzp[:, 0:NH], cw_sb[:, m, 0:1],
                                                    None, ALU.mult)
                            nc.vector.scalar_tensor_tensor(cv[:], xzp[:, 1:1 + NH],
                                                           cw_sb[:, m, 1:2], cv[:],
                                                           ALU.mult, ALU.add)
                            nc.vector.scalar_tensor_tensor(cv[:], xzp[:, 2:2 + NH],
                                                           cw_sb[:, m, 2:3], cv[:],
                                                           ALU.mult, ALU.add)
                            sgc = pt_pool.tile([128, NH], BF, tag="sgc", bufs=2)
                            nc.scalar.activation(sgc[:], cv[:], AF.Sigmoid, bias=cb_sb[:, m:m + 1])
                            nc.vector.scalar_tensor_tensor(xm[m][:], cv[:], cb_sb[:, m:m + 1],
                                                           sgc[:], ALU.add, ALU.mult)

                dtb_sb = load_vec1("dtb", S)
                bpb_sb = load_vec1("bpb", S)
                cpb_sb = load_vec1("cpb", S)
                dbw_sb = pm.tile([128, MB, 128], BF, name="dbw_sb")
                nc.scalar.dma_start(dbw_sb[:], dp["dbw"].ap().rearrange("(kb p) s -> p kb s", p=128))
                cpw_sb = pm.tile([128, MB, S], BF, name="cpw_sb")
                nc.scalar.dma_start(cpw_sb[:], dp["cpw"].ap().rearrange("(kb p) s -> p kb s", p=128))

                # ---- dt/B/C projections + scan ----
                with nc.named_scope("scan"), tc.tile_pool(name="ps3", bufs=1, space="PSUM") as psA:
                    dt_t = pt_pool.tile([S, NH], FP, tag="dt")
                    a_t = pt_pool.tile([S, NH], FP, tag="a")
                    b_t = pt_pool.tile([S, NH], FP, tag="b")
                    c_t = pt_pool.tile([S, NH], FP, tag="c")
                    for n0, nw in ((0, 320), (320, 320)):
                        pzdb = psA.tile([128, 320], FP, tag="pzdb", bufs=2)
                        for kb in range(MB):
                            nc.tensor.matmul(pzdb[:, 0:nw], dbw_sb[:, kb, :],
                                             xm[kb][:, n0:n0 + nw],
                                             start=(kb == 0), stop=(kb == MB - 1))
                        nc.scalar.activation(dt_t[:, n0:n0 + nw], pzdb[0:S, 0:nw],
                                             AF.Sigmoid, bias=dtb_sb[:])
                        nc.vector.scalar_tensor_tensor(b_t[:, n0:n0 + nw], pzdb[S:128, 0:nw],
                                                       bpb_sb[:], dt_t[:, n0:n0 + nw],
                                                       ALU.add, ALU.mult)
                        pzc = psA.tile([S, 320], FP, tag="pzc", bufs=2)
                        for kb in range(MB):
                            nc.tensor.matmul(pzc[:, 0:nw], cpw_sb[:, kb, :],
                                             xm[kb][:, n0:n0 + nw],
                                             start=(kb == 0), stop=(kb == MB - 1))
                        nc.scalar.activation(c_t[:, n0:n0 + nw], pzc[:, 0:nw], AF.Identity,
                                             bias=cpb_sb[:])
                    nc.scalar.activation(a_t[:], dt_t[:], AF.Identity, bias=1.0, scale=-1.0)
                    st_t = pt_pool.tile([S, NH], FP, tag="st")
                    nc.vector.tensor_tensor_scan(st_t[:], a_t[:], b_t[:], 0.0,
                                                 ALU.mult, ALU.add)
                    y_t = pt_pool.tile([S, OWN], FP, tag="dt", name="y_t")
                    nc.vector.tensor_mul(y_t[:], c_t[:, HALO:NH], st_t[:, HALO:NH])

                # ---- layernorm over S (transpose - LN - transpose back) ----
                with nc.named_scope("ln"), tc.tile_pool(name="ps4", bufs=1, space="PSUM") as psA:
                    yln = pt_pool.tile([S, OWN], BF, tag="a", name="yln")
                    for i in range(OTB):
                        ptr = psA.tile([128, 128], FP, tag="ptr", bufs=2)
                        nc.tensor.transpose(ptr[:, 0:S], y_t[:, i * 128:(i + 1) * 128],
                                            ident[0:S, 0:S])
                        yT = pt_pool.tile([128, S], FP, tag="yT", bufs=2)
                        nc.vector.tensor_copy(yT[:], ptr[:, 0:S])
                        mu = pt_pool.tile([128, 1], FP, tag="mu", bufs=2)
                        nc.vector.tensor_reduce(mu[:], yT[:], mybir.AxisListType.X, ALU.add)
                        nc.vector.tensor_scalar_mul(mu[:], mu[:], 1.0 / S)
                        xc = pt_pool.tile([128, S], FP, tag="xc", bufs=2)
                        nc.vector.tensor_scalar_sub(xc[:], yT[:], mu[:])
                        scr2 = pt_pool.tile([128, S], FP, tag="scr2", bufs=2)
                        vv = pt_pool.tile([128, 1], FP, tag="vv", bufs=2)
                        nc.scalar.activation(scr2[:], xc[:], AF.Square, accum_out=vv[:])
                        nc.vector.tensor_scalar(vv[:], vv[:], 1.0 / S, 1e-5, ALU.mult, ALU.add)
                        nc.scalar.sqrt(vv[:], vv[:])
                        nc.vector.reciprocal(vv[:], vv[:])
                        nc.vector.tensor_scalar_mul(xc[:], xc[:], vv[:])
                        ptr2 = psA.tile([128, 128], FP, tag="ptr2", bufs=2)
                        nc.tensor.transpose(ptr2[0:S, :], xc[:], ident[:])
                        nc.vector.tensor_copy(yln[:, i * 128:(i + 1) * 128], ptr2[0:S, :])

                s2ib_sb = load_pcol("s2ib", INNER, 16)
                Dp_sb = load_pcol("Dp", INNER, 16)
                s2iw_sb = pm.tile([S, INNER], BF, name="s2iw_sb")
                nc.scalar.dma_start(s2iw_sb[:], dp["s2iw"][:])

                # ---- s2i + gate sigmoid + pre_out assembly ----
                with nc.named_scope("premix"), tc.tile_pool(name="ps5", bufs=1, space="PSUM") as psA:
                    pre = []
                    for m in range(MB):
                        q, mi = divmod(m, 4)
                        if mi == 0:
                            wq = pt_pool.tile([128, KB, 512], BF, tag="wslab", bufs=2,
                                              name=f"wipg{q}")
                            nc.gpsimd.dma_start(
                                wq[:], dp["ipw"].ap()[:, 2048 + q * 512:2048 + (q + 1) * 512]
                                .rearrange("(kb p) n -> p kb n", p=128))
                        ps = psA.tile([128, 512], FP, tag="ps", bufs=2)
                        nc.tensor.matmul(ps[:], s2iw_sb[:, m * 128:(m + 1) * 128], yln[:],
                                         start=True, stop=True)
                        pg = psA.tile([128, 512], FP, tag="pg", bufs=2)
                        for kb in range(KB):
                            nc.tensor.matmul(pg[:], wq[:, kb, mi * 128:(mi + 1) * 128],
                                             hT[kb][:, HALO:NH],
                                             start=(kb == 0), stop=(kb == KB - 1))
                        sg = pt_pool.tile([128, OWN], FP, tag="sg", bufs=2)
                        nc.scalar.activation(sg[:], pg[:], AF.Sigmoid,
                                             bias=ipb_sb[:, MB + m:MB + m + 1])
                        tmp = pt_pool.tile([128, OWN], FP, tag="tmp", bufs=2)
                        nc.vector.tensor_scalar(tmp[:], xm[m][:, HALO:NH],
                                                Dp_sb[:, m:m + 1], None, ALU.mult)
                        nc.vector.scalar_tensor_tensor(tmp[:], ps[:], s2ib_sb[:, m:m + 1],
                                                       tmp[:], ALU.add, ALU.add)
                        pre_m = pm.tile([128, OWN], BF, tag=f"xm{m}", name=f"pre{m}")
                        nc.vector.tensor_mul(pre_m[:], tmp[:], sg[:])
                        pre.append(pre_m)

                obb_sb = load_row("ob", D, BF)
                gw_sb = po.tile([128, KB, E], FP)  # [p, kb, e]
                nc.scalar.dma_start(gw_sb[:], dp["gw"].ap().rearrange("(kb p) e -> p kb e", p=128))
                gb_sb = load_row("gb", E, FR)
                owsb = [pm.tile([128, D], BF, name=f"owsb{kb}", tag=f"owsb{kb}")
                        for kb in range(MB)]
                for kb in range(MB):
                    nc.scalar.dma_start(owsb[kb][:], dp["ow"][kb * 128:(kb + 1) * 128, :])

                # ---- tb-major: out projection + rms2 + h2T + AG per tb ----
                with nc.named_scope("outgate"), tc.tile_pool(name="ps7", bufs=1, space="PSUM") as psA:
                    for tb in range(OTB):
                        potn = [psA.tile([128, 512], FP, tag=f"pon{nb}", bufs=2,
                                         name=f"pon{nb}_{tb}") for nb in range(2)]
                        for kb in range(MB):
                            for nb in range(2):
                                nc.tensor.matmul(potn[nb][:],
                                                 pre[kb][:, tb * 128:(tb + 1) * 128],
                                                 owsb[kb][:, nb * 512:(nb + 1) * 512],
                                                 start=(kb == 0), stop=False)
                        for nb in range(2):
                            nc.tensor.matmul(potn[nb][:], ones1b[:],
                                             obb_sb[:, nb * 512:(nb + 1) * 512],
                                             start=False, stop=True)
                            nc.vector.tensor_add(xmid[tb][:, nb * 512:(nb + 1) * 512],
                                                 potn[nb][:],
                                                 xo[tb][:, nb * 512:(nb + 1) * 512])
                        scr = pt_pool.tile([128, D], FP, tag="scr", bufs=2)
                        sq = pt_pool.tile([128, 1], FP, tag="sq2", bufs=2)
                        nc.scalar.activation(scr[:], xmid[tb][:], AF.Square, accum_out=sq[:])
                        nr = pt_pool.tile([128, 1], FP, tag="nr2", bufs=2)
                        nc.vector.tensor_scalar(nr[:], sq[:], 1.0 / D, 1e-6, ALU.mult, ALU.add)
                        nc.scalar.sqrt(nr[:], nr[:])
                        nc.vector.reciprocal(nr[:], nr[:])
                        h2 = pt_pool.tile([128, D], FP, tag="xt", bufs=2, name="h2")
                        nc.vector.tensor_scalar(h2[:], xmid[tb][:], nr[:], None, ALU.mult)
                        pl = psA.tile([128, E], FP, tag="pl", bufs=2)
                        for kb in range(KB):
                            ptr = psA.tile([128, 128], FP, tag="ptr", bufs=2)
                            nc.tensor.transpose(ptr[:], h2[:, kb * 128:(kb + 1) * 128], ident[:])
                            h2T_t = pt_pool.tile([128, 128], FP, tag="h2T", bufs=2)
                            nc.vector.tensor_copy(h2T_t[:], ptr[:])
                            nc.tensor.matmul(pl[:], h2T_t[:], gw_sb[:, kb, :],
                                             start=(kb == 0), stop=False)
                        nc.tensor.matmul(pl[:], ones1[:], gb_sb[:], start=False, stop=True)
                        # top-2-of-4 gating
                        m1 = pt_pool.tile([128, 1], FP, tag="m1", bufs=2)
                        nc.vector.tensor_reduce(m1[:], pl[:], mybir.AxisListType.X, ALU.max)
                        eq1 = pt_pool.tile([128, E], FP, tag="eq1", bufs=2)
                        nc.vector.tensor_scalar(eq1[:], pl[:], m1[:], None, ALU.is_equal)
                        msk = pt_pool.tile([128, E], FP, tag="msk", bufs=2)
                        nc.vector.scalar_tensor_tensor(msk[:], eq1[:], -1e30, pl[:],
                                                       ALU.mult, ALU.add)
                        m2 = pt_pool.tile([128, 1], FP, tag="m2", bufs=2)
                        nc.vector.tensor_reduce(m2[:], msk[:], mybir.AxisListType.X, ALU.max)
                        eq2 = pt_pool.tile([128, E], FP, tag="eq2", bufs=2)
                        nc.vector.tensor_scalar(eq2[:], msk[:], m2[:], None, ALU.is_equal)
                        dd = pt_pool.tile([128, 1], FP, tag="dd", bufs=2)
                        nc.vector.tensor_sub(dd[:], m2[:], m1[:])
                        p2 = pt_pool.tile([128, 1], FP, tag="p2", bufs=2)
                        nc.scalar.activation(p2[:], dd[:], AF.Sigmoid)
                        p1b = pt_pool.tile([128, 1], FP, tag="p1b", bufs=2)
                        nc.scalar.activation(p1b[:], p2[:], AF.Identity, bias=1.0, scale=-1.0)
                        nc.vector.tensor_scalar(wv_sb[tb][:], eq1[:], p1b[:], None, ALU.mult)
                        nc.vector.scalar_tensor_tensor(wv_sb[tb][:], eq2[:], p2[:], wv_sb[tb][:],
                                                       ALU.mult, ALU.add)
                        # pack h2 + gate weights into one AllGather payload row
                        h2b = pt_pool.tile([128, DW], BF, tag="h2b", bufs=2)
                        nc.vector.tensor_copy(h2b[:, 0:D], h2[:])
                        nc.vector.tensor_copy(h2b[:, D:D + E], wv_sb[tb][:])
                        nc.vector.memset(h2b[:, D + E:DW], 0.0)
                        gin = gth_in_a if tb < 2 else gth_in_b
                        nc.sync.dma_start(gin[(tb % 2) * 128:(tb % 2 + 1) * 128, :], h2b[:])
                        if tb == 1:
                            nc.gpsimd.collective_compute(
                                "AllGather", ALU.bypass, replica_groups=rg,
                                ins=[gth_in_a.opt()],
                                outs=[gth_all[0:1024, :].opt()])
                    with nc.named_scope("gather"):
                        nc.gpsimd.collective_compute(
                            "AllGather", ALU.bypass, replica_groups=rg,
                            ins=[gth_in_b.opt()],
                            outs=[gth_all[1024:2048, :].opt()])

            # =======================================================
            # MoE (full expert per core, token-half group of 4)
            # =======================================================
            with (
                tc.tile_pool(name="moe", bufs=1) as pq,
                tc.tile_pool(name="psC", bufs=1, space="PSUM") as psC,
            ):
                HB = 2 * HH // 128  # 32 hid blocks
                with nc.named_scope("moe_w"):
                    # ew1 on the scalar queue: round-0 matmuls only need kb=0
                    # so compute starts while the rest streams in.
                    ew1_sb = [pq.tile([128, 2 * HH], BF, name=f"ew1_{kb}", tag=f"ew1_{kb}")
                              for kb in range(KB)]
                    for kb in range(KB):
                        nc.scalar.dma_start(ew1_sb[kb][:], dp["ew1"][kb * 128:(kb + 1) * 128, :])

                with nc.named_scope("moe"):
                    NP = 320          # padded selected-token count per quarter
                    NPB = (NP + 127) // 128   # compact token blocks (128,128,64)
                    PAYR = 384        # pay buffer rows (readback block granularity)
                    ones4 = pq.tile([128, 4], FP)
                    nc.vector.memset(ones4[:], 1.0)

                    # -------- per-round index build (scatter-compact) --------
                    wcomp_r, loc16_r, idxi_r = [], [], []
                    for r in range(4):
                        wvr = pq.tile([128, OTB, E], BF, tag="wvr", bufs=2)
                        nc.sync.dma_start(
                            wvr[:, 0:2, :], gth_all[r * 256:(r + 1) * 256, D:D + E]
                            .rearrange("(t p) e -> p t e", p=128))
                        nc.sync.dma_start(
                            wvr[:, 2:4, :],
                            gth_all[1024 + r * 256:1024 + (r + 1) * 256, D:D + E]
                            .rearrange("(t p) e -> p t e", p=128))
                        wsall = pq.tile([128, 4], FP, tag="wsall", bufs=2)
                        for tb in range(OTB):
                            wm_t = pq.tile([128, E], FP, tag="wm", bufs=2)
                            nc.vector.tensor_mul(wm_t[:], wvr[:, tb, :], esel[:])
                            nc.vector.tensor_reduce(wsall[:, tb:tb + 1], wm_t[:],
                                                    mybir.AxisListType.X, ALU.add)
                        msk = pq.tile([128, 4], FP, tag="msk", bufs=2)
                        nc.vector.tensor_scalar(msk[:], wsall[:], 0.0, None, ALU.is_gt)
                        csum = pq.tile([128, 4], FP, tag="csum", bufs=2)
                        nc.vector.tensor_tensor_scan(csum[:], ones4[:], msk[:], 0.0,
                                                     ALU.mult, ALU.add)
                        pbase = psC.tile([128, 1], FP, tag="ph", bufs=2, name="pbase")
                        nc.tensor.matmul(pbase[:], ltm[:], csum[:, 3:4],
                                         start=True, stop=True)
                        pos = pq.tile([128, 4], FP, tag="pos", bufs=2)
                        nc.vector.tensor_sub(pos[:], csum[:], msk[:])
                        nc.vector.tensor_scalar(pos[:], pos[:], pbase[:], None, ALU.add)
                        dpos = pq.tile([128, 4], FP, tag="dpos", bufs=2)
                        nc.vector.tensor_scalar(dpos[:], pos[:], -4096.0, None, ALU.add)
                        nc.vector.tensor_mul(dpos[:], dpos[:], msk[:])
                        nc.vector.tensor_scalar(dpos[:], dpos[:], 4096.0, None, ALU.add)
                        posi = pq.tile([128, 4], mybir.dt.int32, tag="posi", bufs=2)
                        nc.vector.tensor_copy(posi[:], dpos[:])
                        for tb in range(OTB):
                            payt = pq.tile([128, 4], FP, tag="payt", bufs=2)
                            nc.vector.tensor_copy(payt[:, 0:1], iotaG[:, r * 4 + tb:r * 4 + tb + 1])
                            nc.vector.tensor_copy(payt[:, 1:2], iotaL[:, tb:tb + 1])
                            nc.vector.tensor_copy(payt[:, 2:3], wsall[:, tb:tb + 1])
                            nc.vector.memset(payt[:, 3:4], 0.0)
                            nc.gpsimd.indirect_dma_start(
                                out=pay[r][:], out_offset=bass.IndirectOffsetOnAxis(
                                    ap=posi[:, tb:tb + 1], axis=0),
                                in_=payt[:], in_offset=None,
                                bounds_check=NP - 1, oob_is_err=False)
                        # readbacks (idx first — gates the token gather)
                        idxf = pq.tile([128, PAYR // 16], FP, tag="idxf", bufs=2)
                        for g in range(8):
                            nc.sync.dma_start(
                                idxf[g * 16:(g + 1) * 16, :],
                                pay[r][:, 0:1].rearrange("(c p) o -> p (c o)", p=16))
                        idxi = pq.tile([128, PAYR // 16], mybir.dt.int16, tag=f"idxi{r}",
                                       bufs=1, name=f"idxi{r}")
                        nc.vector.tensor_copy(idxi[:], idxf[:])
                        if r == 0:
                            h2g0 = pq.tile([128, KB, PAYR], BF, tag="h2g0", bufs=1,
                                           name="h2g0")
                            nc.gpsimd.dma_gather(
                                h2g0[:], gth_all[:, 0:D], idxi[:], PAYR, PAYR,
                                elem_size=D, elem_step=DW, transpose=True)
                        wcf = pq.tile([128, NPB], FP, tag=f"wcf{r}", bufs=1, name=f"wcf{r}")
                        nc.sync.dma_start(
                            wcf[:], pay[r][:, 2:3].rearrange("(b p) o -> p (b o)", p=128))
                        locf = pq.tile([128, NPB], FP, tag="locf", bufs=2)
                        nc.sync.dma_start(
                            locf[:], pay[r][:, 1:2].rearrange("(b p) o -> p (b o)", p=128))
                        loci = pq.tile([128, NPB], mybir.dt.int32, tag=f"loci{r}",
                                       bufs=1, name=f"loci{r}")
                        nc.vector.tensor_copy(loci[:], locf[:])
                        idxi_r.append(idxi)
                        wcomp_r.append(wcf)
                        loc16_r.append(loci)

                    # ew2 resident for all rounds (sync queue, after the builds)
                    ew2R = [pq.tile([128, HB, 512], F8, name=f"ew2R{nb}", tag=f"ew2R{nb}")
                            for nb in range(2)]
                    for nb in range(2):
                        nc.sync.dma_start(
                            ew2R[nb][:], dp["ew2"].ap()[:, nb * 512:(nb + 1) * 512]
                            .rearrange("(hb p) d -> p hb d", p=128))

                    # -------- residual prefill for all rounds (bf16) --------
                    # own-expert quarter carries xmid, others zero; done here so
                    # the compute rounds never wait on it.
                    for r in range(4):
                        for tb in range(OTB):
                            xmr = pq.tile([128, D], BF, tag="xmr", bufs=2)
                            nc.vector.tensor_scalar(xmr[:], xmid[tb][:], rmask[:, r:r + 1],
                                                    None, ALU.mult)
                            nc.scalar.dma_start(rs_in[r][tb * 128:(tb + 1) * 128, :], xmr[:])

                    # -------- per-round compute on compacted tokens --------
                    h2g_t = [h2g0, None, None, None]
                    for r in range(4):
                        if r < 3:  # prefetch next round's tokens during this one
                            nxt = pq.tile([128, KB, PAYR], BF, tag="h2g", bufs=2,
                                          name=f"h2g{r + 1}")
                            nc.gpsimd.dma_gather(
                                nxt[:], gth_all[:, 0:D], idxi_r[r + 1][:], PAYR, PAYR,
                                elem_size=D, elem_step=DW, transpose=True)
                            h2g_t[r + 1] = nxt
                        h2g = h2g_t[r]
                        hidp = [pq.tile([128, 2, NP], F8, tag=f"hidp{p}", bufs=1,
                                        name=f"hidp{p}") for p in range(HB // 2)]
                        for h in range(HB):
                            ph = psC.tile([128, NP], FP, tag="ph", bufs=2)
                            for kb in range(KB):
                                nc.tensor.matmul(ph[:], ew1_sb[kb][:, h * 128:(h + 1) * 128],
                                                 h2g[:, kb, 0:NP], start=(kb == 0),
                                                 stop=(kb == KB - 1))
                            nc.scalar.activation(hidp[h // 2][:, h % 2, :], ph[:],
                                                 AF.Gelu, bias=eb1_sb[:, h:h + 1])
                        peo = [[psC.tile([128, 512], FP, tag=f"peo{b}n{nb}", bufs=1,
                                         name=f"peo{b}n{nb}") for nb in range(2)]
                               for b in range(NPB)]
                        wfa = pq.tile([128, NPB, D], BF, tag="wfa", bufs=2)
                        for nb in range(2):
                            for p in range(16):
                                for b in range(NPB):
                                    msz = min(128, NP - b * 128)
                                    nc.tensor.matmul(
                                        peo[b][nb][0:msz, :],
                                        hidp[p][:, :, b * 128:b * 128 + msz],
                                        ew2R[nb][:, 2 * p:2 * p + 2, :],
                                        start=(p == 0), stop=False, perf_mode=DR)
                            for b in range(NPB):
                                msz = min(128, NP - b * 128)
                                nc.tensor.matmul(peo[b][nb][0:msz, :], ones1b[:, 0:msz],
                                                 eb2h_sb[:, nb * 512:(nb + 1) * 512],
                                                 start=False, stop=True)
                        for b in range(NPB):
                            msz = min(128, NP - b * 128)
                            for nb in range(2):
                                nc.vector.tensor_scalar(
                                    wfa[0:msz, b, nb * 512:(nb + 1) * 512],
                                    peo[b][nb][0:msz, :], wcomp_r[r][0:msz, b:b + 1],
                                    None, ALU.mult)
                        for b in range(NPB):
                            nc.gpsimd.indirect_dma_start(
                                out=rs_in[r][:], out_offset=bass.IndirectOffsetOnAxis(
                                    ap=loc16_r[r][:, b:b + 1], axis=0),
                                in_=wfa[:, b, :], in_offset=None,
                                bounds_check=OWN - 1, oob_is_err=False,
                                compute_op=ALU.add)
                        nc.gpsimd.collective_compute(
                            "ReduceScatter", ALU.add, replica_groups=rg,
                            ins=[rs_in[r].opt()], outs=[rs_out[r].opt()])
                        nc.sync.dma_start(out_d[r * 128:(r + 1) * 128, :], rs_out[r][:])

    nc.compile()
    return nc


def host_prep(inputs):
    """Build the 8 per-core input maps from full inputs."""
    import ml_dtypes
    f32 = np.float32
    bf16 = ml_dtypes.bfloat16
    x = np.ascontiguousarray(np.asarray(inputs["x"], f32).reshape(B * T, D))
    n1 = np.asarray(inputs["norm1_w"], f32)
    n2 = np.asarray(inputs["norm2_w"], f32)
    ipw = np.ascontiguousarray(
        (np.asarray(inputs["in_proj_w"], f32) * n1[:, None]).astype(bf16))
    gw = np.ascontiguousarray(np.asarray(inputs["gate_w"], f32) * n2[:, None])
    import ml_dtypes as mld
    ew1f = np.asarray(inputs["e_w1"], f32) * n2[None, :, None]
    ew1b = ew1f.astype(bf16)
    ew2f = np.asarray(inputs["e_w2"], f32)
    # per-expert power-of-2 scale into fp8 e4m3 range (max ~240)
    s2 = np.array([2.0 ** np.floor(np.log2(192.0 / max(np.abs(ew2f[e]).max(), 1e-9)))
                   for e in range(E)], f32)
    ew2q = np.stack([(ew2f[e] * s2[e]).astype(mld.float8_e4m3) for e in range(E)])
    dbw = np.ascontiguousarray(np.concatenate(
        [np.asarray(inputs["dt_w"], f32), np.asarray(inputs["bp_w"], f32)],
        axis=1).astype(bf16))
    ident = np.eye(128, dtype=f32)
    ones1 = np.ones((1, 128), f32)
    pp = np.arange(128, dtype=f32)
    iotaG = np.empty((128, 16), f32)
    for r_ in range(4):
        for tb_ in range(4):
            iotaG[:, r_ * 4 + tb_] = (tb_ // 2) * 1024 + r_ * 256 + (tb_ % 2) * 128 + pp
    iotaL = np.empty((128, 4), f32)
    for tb_ in range(4):
        iotaL[:, tb_] = tb_ * 128 + pp
    ltmat = (pp[:, None] < pp[None, :]).astype(f32)
    pre12 = np.tile(np.array([0.0, 4095.0, 0.0, 0.0], f32), 3)[None, :].repeat(128, 0)
    shared = {
        "ipw": ipw, "ipb": np.asarray(inputs["in_proj_b"], f32),
        "cw": np.ascontiguousarray(np.asarray(inputs["conv_w"], f32)[:, 0, :]),
        "cb": np.asarray(inputs["conv_b"], f32),
        "dbw": dbw,
        "dtb": np.asarray(inputs["dt_b"], f32), "bpb": np.asarray(inputs["bp_b"], f32),
        "cpw": np.asarray(inputs["cp_w"], f32).astype(bf16),
        "cpb": np.asarray(inputs["cp_b"], f32),
        "s2iw": np.asarray(inputs["s2i_w"], f32).astype(bf16),
        "s2ib": np.asarray(inputs["s2i_b"], f32),
        "Dp": np.asarray(inputs["D_param"], f32),
        "ow": np.asarray(inputs["out_w"], f32).astype(bf16),
        "ob": np.asarray(inputs["out_b"], f32).astype(bf16),
        "gw": gw, "gb": np.asarray(inputs["gate_b"], f32),
        "ident": ident, "ones1": ones1, "ones1b": ones1.astype(bf16),
        "iotaG": iotaG, "iotaL": iotaL, "ltmat": np.ascontiguousarray(ltmat),
        "pre12": np.ascontiguousarray(pre12),
    }
    eb1 = np.asarray(inputs["e_b1"], f32)
    eb2 = np.asarray(inputs["e_b2"], f32)
    in_maps = []
    for c in range(N_CORES):
        e, th = c // 2, c % 2
        g0 = th * (B * T // 2) + e * OWN
        if e == 0:
            x_sh = np.concatenate([np.zeros((HALO, D), f32), x[g0:g0 + OWN]])
        else:
            x_sh = x[g0 - HALO:g0 + OWN]
        m = dict(shared)
        m["x_sh"] = np.ascontiguousarray(x_sh)
        m["ew1"] = np.ascontiguousarray(ew1b[e])
        m["eb1"] = np.ascontiguousarray(eb1[e])
        m["ew2"] = np.ascontiguousarray(ew2q[e])
        m["eb2h"] = np.ascontiguousarray((eb2[e] * s2[e]).astype(bf16))
        esel = np.zeros((128, E), f32)
        esel[:, e] = 1.0 / s2[e]  # dequant of fp8-scaled ew2 folded into combine weight
        m["esel"] = esel
        rmask = np.zeros((128, 4), f32)
        rmask[:, e] = 1.0
        m["rmask"] = rmask
        in_maps.append(m)
    return in_maps


def unshard_out(results):
    """results: list of 8 dicts with 'out' [OWN, D]; rows r*128+i of core c
    hold global token (c%2)*2048 + r*512 + (c//2)*128 + i."""
    full = np.empty((B * T, D), np.float32)
    for c in range(N_CORES):
        e, th = c // 2, c % 2
        oc = results[c]["out"]
        for r in range(4):
            full[th * 2048 + r * OWN + e * 128: th * 2048 + r * OWN + (e + 1) * 128] = \
                oc[r * 128:(r + 1) * 128]
    return full.reshape(B, T, D)


_NC_CACHE = {}


def _get_nc():
    if "nc" not in _NC_CACHE:
        _NC_CACHE["nc"] = build(debug_outputs=False)
    return _NC_CACHE["nc"]


def kernel(**inputs) -> np.ndarray:
    """Full-input entry point: shards across 8 NeuronCores, runs the Bass
    kernel SPMD, reassembles the full [2, 2048, 1024] output."""
    import sys, types
    try:  # NTFF profile hook shim (missing antenv.axon_hooks in this image)
        import antenv.axon_hooks  # noqa: F401
    except ImportError:
        try:
            import antenv
            from trn_agent_boot.trn_boot import _ntff_profile_via_ctypes
            mod = types.ModuleType("antenv.axon_hooks")
            try:
                _hook = _ntff_profile_via_ctypes("/opt/axon/libaxon_pjrt.so")
            except Exception:
                _hook = None
            mod.get_axon_ntff_profile_hook = lambda: _hook
            mod.set_axon_ntff_profile_hook = lambda h: None
            sys.modules["antenv.axon_hooks"] = mod
            antenv.axon_hooks = mod
        except Exception:
            pass
    from concourse.bass_utils import run_bass_kernel_spmd

    nc = _get_nc()
    in_maps = host_prep(inputs)
    res = run_bass_kernel_spmd(nc, in_maps, core_ids=list(range(N_CORES)))
    out = unshard_out(res.results)
    return out.astype(np.float32)

